# revision 2
# baseline (speedup 1.0000x reference)
"""AttentionRNNCell Trainium2 kernel (v3).

Math (per batch row b):
  et[t]  = V_a . tanh( (h W_a + b_a) + x[t] U_a )        t in [0, TE)
  at     = exp(et);  s = sum(at)
  ctx    = (sum_t at[t] x[t]) / s
  zt     = sigmoid(h W_z + [inp, ctx] C_z + b_z)
  rt     = sigmoid(h W_r + [inp, ctx] C_r + b_r)
  tht    = tanh((rt*h) U_p + [inp, ctx] C_p + b_p)
  ht     = (1-zt)*h + zt*tht
Distribution: data-parallel over batch B=128 across 8 cores (16 rows each).
Host ships x twice in fp8, pre-tiled in both layouts the PE needs
(xnat: t on partitions for ctx; xtr: e on partitions for uxpb), and folds
everything not depending on x_seq into small per-core tensors.

v3 vs v2 (trace-driven):
  - DMA queues rebalanced (sync: xtr rows 1-15 + gate weights + stores;
    gpsimd: xnat + tail weights; scalar: only ua/va/wxpb, issued first) and
    strictly row-ordered so row-0-critical bytes are never sharing SDMA
    engines with prefetch. Baseline lost ~10us to this at startup.
  - 8 warmup matmuls on a memset ones tile run during the initial DMA wait
    so the PE HAM clock-gate reaches 8/8 before real work (baseline ran the
    first 38us at 1.2GHz).
  - uxpb PSUM tiles are per (t-half, u-chunk) with bufs=2 (4 banks), freeing
    2 banks for a tail pool; the buf recycling self-synchronizes the PE to
    the ACT tanh pace with no ACT idle.
  - exp keeps accum-free: expsum comes from a DVE tensor_reduce of the at2
    diagonal slot instead of ACT ACTIVATION_READ_ACCUMULATOR (-4.8us ACT).
  - tail computed per half-batch (rows 0-7 pipelined into iterations 11-13,
    rows 8-15 after the loop); sigmoid = 1/(1+exp(-x)) on DVE so only one
    ACT table set is ever loaded.
  - PE issue order per iteration: [uxpb-th0(b), et(b-1)(+exp), uxpb-th1(b),
    ctx(pair)] so no PE instruction waits on an ACT result without a full
    stage of PE work ahead of it.
"""

from contextlib import ExitStack

import numpy as np
import ml_dtypes

import concourse.bass as bass
import concourse.mybir as mybir
import concourse.tile as tile

BF16 = ml_dtypes.bfloat16
NPF8 = ml_dtypes.float8_e4m3
F32 = mybir.dt.float32
F8 = mybir.dt.float8e4
DR = mybir.MatmulPerfMode.DoubleRow
AF = mybir.ActivationFunctionType
AX = mybir.AxisListType
ALU = mybir.AluOpType

B, TE, U, IN_DIM = 128, 2048, 256, 256
N_CORES = 8
BS = B // N_CORES  # 16 batch rows per core
P = 128
EC = U // P  # e-chunks (2)
UC = U // P  # u-chunks (2)
HB = BS // 2  # rows per tail half (8)


def split_multi_waits(nc, max_waits=1):
    """This container's walrus rejects instructions carrying more than one
    sync wait. Hoist extra waits onto standalone same-engine NoOps inserted
    immediately before the offending instruction (semantically identical:
    the engine blocks on each wait in order before executing it)."""
    n_new = 0
    for f in nc.m.functions:
        for blk in f.blocks:
            new_insts = []
            for inst in blk.instructions:
                si = inst.sync_info
                waits = list(si.on_wait) if si and si.on_wait else []
                if len(waits) > max_waits:
                    for w in waits[:-max_waits]:
                        nop = mybir.InstNoOp(
                            name=f"{inst.name}-hw{n_new}", ins=[], outs=[]
                        )
                        nop.engine = inst.engine
                        nop.sync_info = mybir.SyncInfo(on_wait=[w], on_update=[])
                        new_insts.append(nop)
                        n_new += 1
                    si.on_wait = waits[-max_waits:]
                new_insts.append(inst)
            blk.instructions = new_insts
    return n_new


def build_nc(bs=BS, te=TE, split_waits=True):
    tc_n = te // P      # 128-col t-chunks (16)
    th_n = 2            # t halves (uxpb PSUM tile = [128, te/2] fp32, 2 banks)
    t_half = te // th_n
    tq_n = t_half // P  # 128-col chunks per half (8)
    n_mm = min(512, t_half)

    nc = bass.Bass()
    xnat_d = nc.declare_dram_parameter("xnat", [bs // 2, P, tc_n, 2, U], F8, isOutput=False)
    xtr_d = nc.declare_dram_parameter("xtr", [bs, P, EC, te], F8, isOutput=False)
    ua_d = nc.declare_dram_parameter("ua", [U, U], F8, isOutput=False)
    va_d = nc.declare_dram_parameter("va", [U, 1], F8, isOutput=False)
    wxpbT_d = nc.declare_dram_parameter("wxpbT", [U, bs], F32, isOutput=False)
    hT_d = nc.declare_dram_parameter("hT", [U, bs], F32, isOutput=False)
    g0T_d = nc.declare_dram_parameter("g0T", [3, U, bs], F32, isOutput=False)
    cz_d = nc.declare_dram_parameter("cz", [U, U], F32, isOutput=False)
    cr_d = nc.declare_dram_parameter("cr", [U, U], F32, isOutput=False)
    cp_d = nc.declare_dram_parameter("cp", [U, U], F32, isOutput=False)
    up_d = nc.declare_dram_parameter("up", [U, U], F32, isOutput=False)
    id_d = nc.declare_dram_parameter("ident", [P, P], F32, isOutput=False)
    ht_d = nc.declare_dram_parameter("ht", [bs, U], F32, isOutput=True)

    with tile.TileContext(nc) as tc, ExitStack() as ctx:
        singles = ctx.enter_context(tc.tile_pool(name="singles", bufs=1))
        xnat_p = ctx.enter_context(tc.tile_pool(name="xnat", bufs=4))
        xtr_p = ctx.enter_context(tc.tile_pool(name="xtr", bufs=3))
        tanh_p = ctx.enter_context(tc.tile_pool(name="tanh", bufs=8))
        small_p = ctx.enter_context(tc.tile_pool(name="small", bufs=4))
        uxpb_ps = ctx.enter_context(tc.tile_pool(name="uxpbps", bufs=2, space="PSUM"))
        et_ps = ctx.enter_context(tc.tile_pool(name="etps", bufs=1, space="PSUM"))
        ctx_ps = ctx.enter_context(tc.tile_pool(name="ctxps", bufs=1, space="PSUM"))
        tail_ps = ctx.enter_context(tc.tile_pool(name="tailps", bufs=2, space="PSUM"))

        # ---- weights / small per-core tensors ----
        ua_sb = singles.tile([P, EC, U], F8)
        va_sb = singles.tile([P, UC, 1], F8)
        wxpb_sb = singles.tile([P, UC, bs], F32)

        def load_first_weights():
            # Scalar HWDGE ring: tiny, lands ~1us after issue, never competes
            # with the bulk x traffic on the sync/gpsimd rings.
            nc.scalar.dma_start(out=ua_sb, in_=ua_d[:, :].rearrange("(c p) u -> p c u", p=P))
            nc.scalar.dma_start(out=va_sb, in_=va_d[:, :].rearrange("(c p) o -> p c o", p=P))
            nc.scalar.dma_start(out=wxpb_sb, in_=wxpbT_d[:, :].rearrange("(c p) b -> p c b", p=P))

        hT_sb = singles.tile([P, UC, bs], F32)
        g0_sb = singles.tile([P, 3, UC, bs], F32)
        gate_w = {}
        for name in ("cz", "cr", "cp", "up"):
            gate_w[name] = singles.tile([P, EC, U], F32, name=f"{name}_sb")
        id_sb = singles.tile([P, P], F32)
        ones_sb = singles.tile([P, P], F32)
        nc.vector.memset(ones_sb, 1.0)
        ones8 = singles.tile([P, 512], F8)
        nc.vector.memset(ones8, 1.0)

        def load_tail_weights():
            # gpsimd ring, after xnat pair 0: needed only from ~iteration 11.
            nc.gpsimd.dma_start(out=hT_sb, in_=hT_d[:, :].rearrange("(c p) b -> p c b", p=P))
            nc.gpsimd.dma_start(out=g0_sb, in_=g0T_d[:, :, :].rearrange("g (c p) b -> p g c b", p=P))
            nc.gpsimd.dma_start(out=id_sb, in_=id_d[:, :])

        def load_gate_weights():
            # sync ring, after row 4's xtr: needed from ~iteration 12.
            for name, d in (("cz", cz_d), ("cr", cr_d), ("cp", cp_d), ("up", up_d)):
                nc.sync.dma_start(out=gate_w[name], in_=d[:, :].rearrange("(c p) u -> p c u", p=P))

        expsum_all = singles.tile([P, bs], F32)
        # Unnormalized per-pair ctx rows land here via tiny SBUF->SBUF DMAs
        # (cross-partition move). Separate tile per tail half so the tail
        # transposes read partitions 0-7 only.
        ctx_rows = [singles.tile([HB, U], F32, name=f"ctx_rows{h}") for h in range(2)]
        ctxn = singles.tile([P, EC, bs], F32)  # normalized ctx^T [e%128, ec, b]
        # Block-diagonal at tiles for the paired-ctx DoubleRow: slot [j, m]
        # holds row (2q+j)'s at iff j == m, else stays the zero written once
        # here. Two tiles ping-pong across pairs. [p, j, tc, m] layout: the
        # k-tile (j) stride is tc_n*2 bytes (dual-fp8 ldweights needs >=16B).
        at2_tiles = []
        for i in range(2):
            at2 = singles.tile([P, 2, tc_n, 2], F8, name=f"at2_{i}")
            nc.vector.memset(at2, 0.0)
            at2_tiles.append(at2)

        # ---- HAM warmup: ~3.4us of dummy matmuls during the initial DMA
        # wait so the PE clock-gate is at 8/8 when row 0's data lands.
        warm = tail_ps.tile([P, 512], F32, tag="tail", name="warm")
        for _ in range(8):
            nc.tensor.matmul(out=warm, lhsT=ones8[:, 0:P], rhs=ones8)

        # ---- streaming stages ----
        def stage_dma(b):
            x_nat = None
            if b % 2 == 0:  # x for ctx lands pair-interleaved, one tile per pair
                x_nat = xnat_p.tile([P, tc_n, 2, U], F8, tag="xnat", name=f"xnat{b}")
                nc.gpsimd.dma_start(out=x_nat, in_=xnat_d[b // 2])
            if b == 0:
                # Row 0 only: four one-shot quarter tiles split across both
                # bulk rings; each first-row uxpb matmul depends on exactly
                # one small DMA, and both rings lead with row-0 bytes.
                xt = [
                    singles.tile([P, EC, 512], F8, name=f"xt0_{c}")
                    for c in range(4)
                ]
                engs = [nc.sync, nc.sync, nc.gpsimd, nc.gpsimd]
                for c in range(4):
                    engs[c].dma_start(
                        out=xt[c], in_=xtr_d[b, :, :, c * 512 : (c + 1) * 512]
                    )
            else:
                xt = xtr_p.tile([P, EC, te], F8, tag="xt", name=f"xt{b}")
                nc.sync.dma_start(out=xt[:, :, 0:t_half], in_=xtr_d[b, :, :, 0:t_half])
                nc.sync.dma_start(out=xt[:, :, t_half:te], in_=xtr_d[b, :, :, t_half:te])
            return x_nat, xt

        def stage_uxpb_th(b, th, xt):
            # uxpb: out[u, t] = sum_e ua[e, u] * xt[e, t] -- fp8 DoubleRow
            # contracts both e-chunks in one matmul. tanh (per-partition
            # bias) -> SBUF fp8 [u, uc, t] tiles for the et DoubleRow.
            tanh_t = tanh_p.tile([P, UC, t_half], F8, tag="tanh", name=f"th{b}_{th}")
            for uc in range(UC):
                ux = uxpb_ps.tile([P, t_half], F32, tag="ux", name=f"ux{b}{th}{uc}")
                for n0 in range(0, t_half, n_mm):
                    if isinstance(xt, list):
                        rhs = xt[(th * t_half + n0) // 512]
                    else:
                        rhs = xt[:, :, th * t_half + n0 : th * t_half + n0 + n_mm]
                    nc.tensor.matmul(
                        out=ux[:, n0 : n0 + n_mm],
                        lhsT=ua_sb[:, :, uc * P : (uc + 1) * P],
                        rhs=rhs,
                        perf_mode=DR,
                    )
                nc.scalar.activation(
                    out=tanh_t[:, uc, :], in_=ux, func=AF.Tanh,
                    bias=wxpb_sb[:, uc, b : b + 1],
                )
            return tanh_t

        def stage_et_exp(b, tanh_ts):
            et = et_ps.tile([P, tc_n], F32, tag="etp", name=f"et{b}")
            for th in range(th_n):
                for tq in range(tq_n):
                    nc.tensor.matmul(
                        out=et[:, th * tq_n + tq : th * tq_n + tq + 1],
                        lhsT=tanh_ts[th][:, :, tq * P : (tq + 1) * P],
                        rhs=va_sb,
                        perf_mode=DR,
                    )
            # exp lands on the diagonal slot of the pair's block-diag tile;
            # expsum via DVE reduce of that slot (no ACT accumulator read).
            j = b % 2
            at2 = at2_tiles[(b // 2) % 2]
            nc.scalar.activation(out=at2[:, j, :, j], in_=et, func=AF.Exp)
            nc.vector.tensor_reduce(
                out=expsum_all[:, b : b + 1], in_=at2[:, j, :, j],
                axis=AX.X, op=ALU.add,
            )
            return at2

        def stage_ctx_pair(q, at2, x_nat):
            # Paired ctx: block-diagonal at2 on the two k-tiles against the
            # pair-interleaved x tile -> out[m, e] = row (2q+m)'s ctx partial.
            cps = ctx_ps.tile([2, U], F32, tag="ctxps", name=f"cps{q}")
            for tcc in range(tc_n):
                nc.tensor.matmul(
                    out=cps,
                    lhsT=at2[:, :, tcc, :],
                    rhs=x_nat[:, tcc, :, :],
                    start=(tcc == 0),
                    stop=(tcc == tc_n - 1),
                    perf_mode=DR,
                )
            stg = small_p.tile([2, U], F32, tag="ctxstg", name=f"stg{q}")
            nc.vector.tensor_copy(stg, cps)
            nc.sync.dma_start(
                out=ctx_rows[q // 4][2 * (q % 4) : 2 * (q % 4) + 2, :], in_=stg
            )

        # ---- tail: per half-batch of 8 rows (h=0 pipelined into the
        # stream at iterations 11-13, h=1 after the loop) ----
        recips_h = [None, None]
        zt_h = [None, None]
        rh_h = [None, None]

        def tail_s_ctx(h):
            rows = slice(HB * h, HB * h + HB)
            s_ps = tail_ps.tile([P, HB], F32, tag="tail", name=f"sps{h}")
            nc.tensor.matmul(out=s_ps, lhsT=ones_sb, rhs=expsum_all[:, rows])
            rec = small_p.tile([P, HB], F32, name=f"recip{h}")
            nc.vector.reciprocal(rec, s_ps)
            recips_h[h] = rec
            for e in range(EC):
                tp = tail_ps.tile([P, HB], F32, tag="tail", name=f"ctxT{h}{e}")
                nc.tensor.transpose(tp, ctx_rows[h][:, e * P : (e + 1) * P], id_sb[0:HB, 0:HB])
                nc.vector.tensor_mul(ctxn[:, e, rows], tp, rec)

        def _sigmoid(dst, g, gi, uc, rows):
            # sigmoid(v) = 1/(1+exp(-v)) -- keeps the tail inside the
            # exp_and_others ACT table set (no second table load).
            tmp = small_p.tile([P, HB], F32, tag="gtmp", name=f"t{gi}{uc}")
            nc.vector.tensor_add(tmp, g, g0_sb[:, gi, uc, rows])
            ex = small_p.tile([P, HB], F32, tag="gtmp", name=f"e{gi}{uc}")
            nc.scalar.activation(out=ex, in_=tmp, func=AF.Exp, scale=-1.0)
            nc.vector.tensor_scalar_add(ex, ex, 1.0)
            nc.vector.reciprocal(dst, ex)

        def tail_zr(h):
            rows = slice(HB * h, HB * h + HB)
            zt = small_p.tile([P, UC, HB], F32, name=f"zt{h}")
            rh = small_p.tile([P, UC, HB], F32, name=f"rh{h}")
            for gi, wname in ((0, "cz"), (1, "cr")):
                for uc in range(UC):
                    g = tail_ps.tile([P, HB], F32, tag="tail", name=f"g{wname}{h}{uc}")
                    for e in range(EC):
                        nc.tensor.matmul(
                            out=g,
                            lhsT=gate_w[wname][:, e, uc * P : (uc + 1) * P],
                            rhs=ctxn[:, e, rows],
                            start=(e == 0),
                            stop=(e == EC - 1),
                        )
                    if gi == 0:
                        _sigmoid(zt[:, uc, :], g, gi, uc, rows)
                    else:
                        rt = small_p.tile([P, HB], F32, tag="gtmp", name=f"rt{h}{uc}")
                        _sigmoid(rt, g, gi, uc, rows)
                        nc.vector.tensor_mul(rh[:, uc, :], rt, hT_sb[:, uc, rows])
            zt_h[h], rh_h[h] = zt, rh

        def tail_p_out(h):
            rows = slice(HB * h, HB * h + HB)
            zt, rh = zt_h[h], rh_h[h]
            stg = small_p.tile([HB, U], F32, name=f"htstg{h}")
            for uc in range(UC):
                g = tail_ps.tile([P, HB], F32, tag="tail", name=f"gp{h}{uc}")
                i = 0
                for w_sb, rhs_fn in (
                    (gate_w["up"], lambda e: rh[:, e, :]),
                    (gate_w["cp"], lambda e: ctxn[:, e, rows]),
                ):
                    for e in range(EC):
                        nc.tensor.matmul(
                            out=g,
                            lhsT=w_sb[:, e, uc * P : (uc + 1) * P],
                            rhs=rhs_fn(e),
                            start=(i == 0),
                            stop=(i == 2 * EC - 1),
                        )
                        i += 1
                tmp = small_p.tile([P, HB], F32, tag="gtmp", name=f"tp{h}{uc}")
                nc.vector.tensor_add(tmp, g, g0_sb[:, 2, uc, rows])
                tht = small_p.tile([P, HB], F32, tag="gtmp", name=f"tht{h}{uc}")
                nc.scalar.activation(out=tht, in_=tmp, func=AF.Tanh)
                # ht^T = h^T + zt^T*(tht^T - h^T)
                nc.vector.tensor_sub(tht, tht, hT_sb[:, uc, rows])
                nc.vector.tensor_mul(tht, tht, zt[:, uc, :])
                nc.vector.tensor_add(tht, tht, hT_sb[:, uc, rows])
                tp = tail_ps.tile([HB, P], F32, tag="tail", name=f"htp{h}{uc}")
                nc.tensor.transpose(tp, tht, id_sb)
                nc.vector.tensor_copy(stg[:, uc * P : (uc + 1) * P], tp)
            nc.sync.dma_start(out=ht_d[rows, :], in_=stg)

        # ---- main loop, software-pipelined one row deep ----
        load_first_weights()
        prev = None  # (b, tanh_ts)
        pair_xnat = {}
        for b in range(bs):
            x_nat, xt = stage_dma(b)
            if x_nat is not None:
                pair_xnat[b // 2] = x_nat
            if b == 1:
                load_tail_weights()
            if b == 4:
                load_gate_weights()
            th0 = stage_uxpb_th(b, 0, xt)
            at2_prev = None
            if prev is not None:
                at2_prev = stage_et_exp(prev[0], prev[1])
                pb = prev[0]
            th1 = stage_uxpb_th(b, 1, xt)
            if prev is not None and pb % 2 == 1:
                stage_ctx_pair(pb // 2, at2_prev, pair_xnat.pop(pb // 2))
            if b == 11:
                tail_s_ctx(0)
            elif b == 12:
                tail_zr(0)
            elif b == 13:
                tail_p_out(0)
            prev = (b, [th0, th1])
        at2_last = stage_et_exp(prev[0], prev[1])
        stage_ctx_pair(prev[0] // 2, at2_last, pair_xnat.pop(prev[0] // 2))
        tail_s_ctx(1)
        tail_zr(1)
        tail_p_out(1)

    if split_waits:
        split_multi_waits(nc)
    return nc


def _host_prep(inputs, h_tm, V_a, W_a, U_a, b_a, C_z, W_z, b_z, C_r, W_r, b_r,
               C_p, U_p, b_p):
    """Fold everything not depending on x_seq into small per-core tensors."""
    wxpb = h_tm @ W_a + b_a                                # [B, U]
    g_z0 = h_tm @ W_z + inputs @ C_z[:IN_DIM] + b_z        # [B, U]
    g_r0 = h_tm @ W_r + inputs @ C_r[:IN_DIM] + b_r
    g_p0 = inputs @ C_p[:IN_DIM] + b_p
    shared = {
        "ua": np.ascontiguousarray(U_a.astype(NPF8)),
        "va": np.ascontiguousarray(V_a.reshape(U, 1).astype(NPF8)),
        "cz": np.ascontiguousarray(C_z[IN_DIM:].astype(np.float32)),
        "cr": np.ascontiguousarray(C_r[IN_DIM:].astype(np.float32)),
        "cp": np.ascontiguousarray(C_p[IN_DIM:].astype(np.float32)),
        "up": np.ascontiguousarray(U_p.astype(np.float32)),
        "ident": np.eye(P, dtype=np.float32),
    }
    per_core = []
    for c in range(N_CORES):
        s = slice(c * BS, (c + 1) * BS)
        per_core.append(
            {
                "wxpbT": np.ascontiguousarray(wxpb[s].T.astype(np.float32)),
                "hT": np.ascontiguousarray(h_tm[s].T.astype(np.float32)),
                "g0T": np.ascontiguousarray(
                    np.stack([g_z0[s].T, g_r0[s].T, g_p0[s].T]).astype(np.float32)
                ),
                **shared,
            }
        )
    return per_core


def _prep_x(x_core):
    """Pre-tile one core's x [bs, TE, U] into both fp8 layouts."""
    xb = x_core.astype(NPF8)
    tc_n = TE // P
    # xnat[q, p, tc, j, e] = x[2q+j, tc*128+p, e]  (pair-interleaved)
    xnat = np.ascontiguousarray(
        xb.reshape(BS // 2, 2, tc_n, P, U).transpose(0, 3, 2, 1, 4)
    )
    # xtr[b, p, ec, t] = x[b, t, ec*128+p]
    xtr = np.ascontiguousarray(
        xb.reshape(BS, TE, EC, P).transpose(0, 3, 2, 1)
    )
    return xnat, xtr


def build_in_maps(all_inputs):
    """Full host prep: dict of the reference's 16 inputs -> per-core in_maps."""
    args = {k: np.asarray(v, dtype=np.float32) for k, v in all_inputs.items()
            if k != "x_seq"}
    x_seq = np.asarray(all_inputs["x_seq"], dtype=np.float32)
    per_core = _host_prep(**args)
    in_maps = []
    for c in range(N_CORES):
        m = dict(per_core[c])
        m["xnat"], m["xtr"] = _prep_x(x_seq[c * BS : (c + 1) * BS])
        in_maps.append(m)
    return in_maps


def kernel(inputs, h_tm, x_seq, V_a, W_a, U_a, b_a, C_z, W_z, b_z,
           C_r, W_r, b_r, C_p, U_p, b_p):
    from concourse.bass_utils import run_bass_kernel_spmd

    in_maps = build_in_maps(dict(
        inputs=inputs, h_tm=h_tm, x_seq=x_seq, V_a=V_a, W_a=W_a, U_a=U_a,
        b_a=b_a, C_z=C_z, W_z=W_z, b_z=b_z, C_r=C_r, W_r=W_r, b_r=b_r,
        C_p=C_p, U_p=U_p, b_p=b_p))
    nc = build_nc()
    res = run_bass_kernel_spmd(nc, in_maps, core_ids=list(range(N_CORES)))
    return np.concatenate([res.results[c]["ht"] for c in range(N_CORES)], axis=0)


# revision 9
# speedup vs baseline: 1.0095x; 1.0095x over previous
"""AttentionRNNCell Trainium2 kernel (v4).

Math (per batch row b):
  et[t]  = V_a . tanh( (h W_a + b_a) + x[t] U_a )        t in [0, TE)
  at     = exp(et);  s = sum(at)
  ctx    = (sum_t at[t] x[t]) / s
  zt     = sigmoid(h W_z + [inp, ctx] C_z + b_z)
  rt     = sigmoid(h W_r + [inp, ctx] C_r + b_r)
  tht    = tanh((rt*h) U_p + [inp, ctx] C_p + b_p)
  ht     = (1-zt)*h + zt*tht
Distribution: data-parallel over batch B=128 across 8 cores (16 rows each).
Host ships x twice in fp8, pre-tiled in both layouts the PE needs
(xnat: t on partitions for ctx; xtr: e on partitions for uxpb), and folds
everything not depending on x_seq into small per-core tensors.

v4 vs v3 (trace-driven):
  - rows 0/1 xtr go first on the gpsimd (SWDGE) ring in half-row DMAs;
    rows 2-15 ship as PAIR tiles in one DMA each (8KB/partition contiguous
    -> 8KB descriptors; the v3 half-row split produced 1KB descriptors and
    the HWDGE ring drained at only ~76GB/s).
  - xtr pair pool bufs=4 (8 rows of lookahead) so the DMA stream runs at
    ring rate instead of being throttled to compute pace by pool WAR deps
    (v3's row 15 data landed at ~100us, stretching the whole kernel).
  - a dummy ACT right after the preamble pulls the one-time ACT table load
    (~2.7us) off the first-tanh critical path.
  - ctx is transposed + staged per PAIR right after its matmul (DVE copy
    of the [2,256] psum, two tiny PE transposes, DVE copies into ctxn) --
    no SBUF->SBUF ctx_rows DMAs, whose ~2.4us completion latency sat on
    the v3 tail.  Normalization by 1/s happens once per half-batch.
  - gate weights / ctxn / rh are bf16 (fast FWL ldweights; v3's fp32 gate
    LDWEIGHTS were 333ns each) and each gate's two u-chunks share one
    [P,2,8] psum tile -> one ACT per gate (3 per half instead of 6 chains).
  - sigmoid = 1/(1+exp(-x)) on DVE keeps the whole kernel inside the one
    exp_and_others ACT table set.
"""

from contextlib import ExitStack

import numpy as np
import ml_dtypes

import concourse.bass as bass
import concourse.mybir as mybir
import concourse.tile as tile

BF16 = ml_dtypes.bfloat16
NPF8 = ml_dtypes.float8_e4m3
F32 = mybir.dt.float32
BF = mybir.dt.bfloat16
F8 = mybir.dt.float8e4
DR = mybir.MatmulPerfMode.DoubleRow
AF = mybir.ActivationFunctionType
AX = mybir.AxisListType
ALU = mybir.AluOpType

B, TE, U, IN_DIM = 128, 2048, 256, 256
N_CORES = 8
BS = B // N_CORES  # 16 batch rows per core
P = 128
EC = U // P  # e-chunks (2)
UC = U // P  # u-chunks (2)
HB = BS // 2  # rows per tail half (8)


def split_multi_waits(nc, max_waits=1):
    """This container's walrus rejects instructions carrying more than one
    sync wait. Hoist extra waits onto standalone same-engine NoOps inserted
    immediately before the offending instruction (semantically identical:
    the engine blocks on each wait in order before executing it)."""
    n_new = 0
    for f in nc.m.functions:
        for blk in f.blocks:
            new_insts = []
            for inst in blk.instructions:
                si = inst.sync_info
                waits = list(si.on_wait) if si and si.on_wait else []
                if len(waits) > max_waits:
                    for w in waits[:-max_waits]:
                        nop = mybir.InstNoOp(
                            name=f"{inst.name}-hw{n_new}", ins=[], outs=[]
                        )
                        nop.engine = inst.engine
                        nop.sync_info = mybir.SyncInfo(on_wait=[w], on_update=[])
                        new_insts.append(nop)
                        n_new += 1
                    si.on_wait = waits[-max_waits:]
                new_insts.append(inst)
            blk.instructions = new_insts
    return n_new


def build_nc(bs=BS, te=TE, split_waits=True):
    tc_n = te // P      # 128-col t-chunks (16)
    th_n = 2            # t halves
    t_half = te // th_n
    tq_n = t_half // P  # 128-col chunks per half (8)
    n_mm = min(512, t_half)

    nc = bass.Bass()
    xnat_d = nc.declare_dram_parameter("xnat", [bs // 2, P, tc_n, 2, U], F8, isOutput=False)
    xtr_d = nc.declare_dram_parameter("xtr", [bs, P, EC, te], F8, isOutput=False)
    ua_d = nc.declare_dram_parameter("ua", [U, U], F8, isOutput=False)
    va_d = nc.declare_dram_parameter("va", [U, 1], F8, isOutput=False)
    wxpbT_d = nc.declare_dram_parameter("wxpbT", [U, bs], F32, isOutput=False)
    hT_d = nc.declare_dram_parameter("hT", [U, bs], F32, isOutput=False)
    g0T_d = nc.declare_dram_parameter("g0T", [3, U, bs], F32, isOutput=False)
    cz_d = nc.declare_dram_parameter("cz", [U, U], BF, isOutput=False)
    cr_d = nc.declare_dram_parameter("cr", [U, U], BF, isOutput=False)
    cp_d = nc.declare_dram_parameter("cp", [U, U], BF, isOutput=False)
    up_d = nc.declare_dram_parameter("up", [U, U], BF, isOutput=False)
    id_d = nc.declare_dram_parameter("ident", [P, P], F32, isOutput=False)
    ht_d = nc.declare_dram_parameter("ht", [bs, U], F32, isOutput=True)

    with tile.TileContext(nc) as tc, ExitStack() as ctx:
        singles = ctx.enter_context(tc.tile_pool(name="singles", bufs=1))
        xnat_p = ctx.enter_context(tc.tile_pool(name="xnat", bufs=4))
        xtr_p = ctx.enter_context(tc.tile_pool(name="xtr", bufs=4))
        tanh_p = ctx.enter_context(tc.tile_pool(name="tanh", bufs=8))
        small_p = ctx.enter_context(tc.tile_pool(name="small", bufs=4))
        uxpb_ps = ctx.enter_context(tc.tile_pool(name="uxpbps", bufs=2, space="PSUM"))
        et_ps = ctx.enter_context(tc.tile_pool(name="etps", bufs=1, space="PSUM"))
        ctx_ps = ctx.enter_context(tc.tile_pool(name="ctxps", bufs=1, space="PSUM"))
        tail_ps = ctx.enter_context(tc.tile_pool(name="tailps", bufs=2, space="PSUM"))

        # ---- weights / small per-core tensors ----
        ua_sb = singles.tile([P, EC, U], F8)
        va_sb = singles.tile([P, UC, 1], F8)
        wxpb_sb = singles.tile([P, UC, bs], F32)

        def load_first_weights():
            # Scalar HWDGE ring: tiny, lands ~1us after issue, never competes
            # with the bulk x traffic on the sync/gpsimd rings.
            nc.scalar.dma_start(out=ua_sb, in_=ua_d[:, :].rearrange("(c p) u -> p c u", p=P))
            nc.scalar.dma_start(out=va_sb, in_=va_d[:, :].rearrange("(c p) o -> p c o", p=P))
            nc.scalar.dma_start(out=wxpb_sb, in_=wxpbT_d[:, :].rearrange("(c p) b -> p c b", p=P))

        hT_sb = singles.tile([P, UC, bs], F32)
        g0_sb = singles.tile([P, 3, UC, bs], F32)
        gate_w = {}
        for name in ("cz", "cr", "cp", "up"):
            gate_w[name] = singles.tile([P, EC, U], BF, name=f"{name}_sb")
        id_sb = singles.tile([P, P], F32)
        idb_sb = singles.tile([P, P], BF)
        ones_sb = singles.tile([P, P], F32)
        nc.vector.memset(ones_sb, 1.0)
        ones8 = singles.tile([P, 512], F8)
        nc.vector.memset(ones8, 1.0)

        def load_tail_weights():
            # gpsimd ring, after xnat pair 1: needed only from ~iteration 11.
            nc.gpsimd.dma_start(out=hT_sb, in_=hT_d[:, :].rearrange("(c p) b -> p c b", p=P))
            nc.gpsimd.dma_start(out=g0_sb, in_=g0T_d[:, :, :].rearrange("g (c p) b -> p g c b", p=P))
            nc.gpsimd.dma_start(out=id_sb, in_=id_d[:, :])
            nc.vector.tensor_copy(idb_sb, id_sb)

        def load_gate_weights():
            # gpsimd ring, after xnat pair 2 (bf16, 128KB each).
            for name, d in (("cz", cz_d), ("cr", cr_d), ("cp", cp_d), ("up", up_d)):
                nc.gpsimd.dma_start(out=gate_w[name], in_=d[:, :].rearrange("(c p) u -> p c u", p=P))

        expsum_all = singles.tile([P, bs], F32)
        # Unnormalized ctx^T columns, staged per pair via tiny PE transposes
        # (no SBUF->SBUF DMA). Normalized in-place per half-batch.
        ctxn = singles.tile([P, EC, bs], BF)
        # Block-diagonal at tiles for the paired-ctx DoubleRow: slot [j, m]
        # holds row (2q+j)'s at iff j == m, else stays the zero written once
        # here. Two tiles ping-pong across pairs. [p, j, tc, m] layout: the
        # k-tile (j) stride is tc_n*2 bytes (dual-fp8 ldweights needs >=16B).
        at2_tiles = []
        for i in range(2):
            at2 = singles.tile([P, 2, tc_n, 2], F8, name=f"at2_{i}")
            nc.vector.memset(at2, 0.0)
            at2_tiles.append(at2)

        # ---- ACT table preload + HAM warmup, both during the initial DMA
        # wait: the weight DMAs issue first on the scalar ring, then a dummy
        # ACT pulls the one-time exp_and_others table load off the
        # first-tanh critical path; ~2.6us of dummy matmuls get the PE
        # clock-gate warming before row 0's data lands.
        load_first_weights()
        actwarm = small_p.tile([P, 1], F32, name="actwarm")
        nc.scalar.activation(out=actwarm, in_=ones_sb[:, 0:1], func=AF.Tanh)
        warm = tail_ps.tile([P, 512], F32, tag="tail", name="warm")
        for _ in range(6):
            nc.tensor.matmul(out=warm, lhsT=ones8[:, 0:P], rhs=ones8)

        # ---- streaming stages ----
        pend_pair = {}

        def stage_dma(b):
            x_nat = None
            if b % 2 == 0:
                x_nat = xnat_p.tile([P, tc_n, 2, U], F8, tag="xnat", name=f"xnat{b}")
                # xnat pair 0 rides the otherwise-idle sync ring so it never
                # competes with rows 0/1 on gpsimd; pair 7 fills sync's tail.
                eng = nc.sync if b in (0, 14) else nc.gpsimd
                eng.dma_start(out=x_nat, in_=xnat_d[b // 2])
            if b < 2:
                # Rows 0/1: half-row DMAs, first in the gpsimd ring, so the
                # first uxpb matmul only waits on a 256KB transfer.
                xt = singles.tile([P, EC, te], F8, name=f"xt{b}")
                nc.gpsimd.dma_start(out=xt[:, :, 0:t_half], in_=xtr_d[b, :, :, 0:t_half])
                nc.gpsimd.dma_start(out=xt[:, :, t_half:te], in_=xtr_d[b, :, :, t_half:te])
            elif b % 2 == 0:
                # One DMA per pair: per-partition-contiguous 2x4KB source
                # blocks -> big descriptors, full HWDGE ring rate.
                xp = xtr_p.tile([P, 2, EC, te], F8, tag="xt", name=f"xt{b}")
                nc.sync.dma_start(
                    out=xp, in_=xtr_d[b : b + 2].rearrange("j p c t -> p j c t")
                )
                pend_pair[b // 2] = xp
                xt = xp[:, 0]
            else:
                xt = pend_pair.pop(b // 2)[:, 1]
            return x_nat, xt

        def stage_uxpb_th(b, th, xt):
            # uxpb: out[u, t] = sum_e ua[e, u] * xt[e, t] -- fp8 DoubleRow
            # contracts both e-chunks in one matmul. tanh (per-partition
            # bias) -> SBUF fp8 [u, uc, t] tiles for the et DoubleRow.
            tanh_t = tanh_p.tile([P, UC, t_half], F8, tag="tanh", name=f"th{b}_{th}")
            for uc in range(UC):
                ux = uxpb_ps.tile([P, t_half], F32, tag="ux", name=f"ux{b}{th}{uc}")
                for n0 in range(0, t_half, n_mm):
                    nc.tensor.matmul(
                        out=ux[:, n0 : n0 + n_mm],
                        lhsT=ua_sb[:, :, uc * P : (uc + 1) * P],
                        rhs=xt[:, :, th * t_half + n0 : th * t_half + n0 + n_mm],
                        perf_mode=DR,
                    )
                nc.scalar.activation(
                    out=tanh_t[:, uc, :], in_=ux, func=AF.Tanh,
                    bias=wxpb_sb[:, uc, b : b + 1],
                )
            return tanh_t

        def stage_et_exp(b, tanh_ts):
            et = et_ps.tile([P, tc_n], F32, tag="etp", name=f"et{b}")
            for th in range(th_n):
                for tq in range(tq_n):
                    nc.tensor.matmul(
                        out=et[:, th * tq_n + tq : th * tq_n + tq + 1],
                        lhsT=tanh_ts[th][:, :, tq * P : (tq + 1) * P],
                        rhs=va_sb,
                        perf_mode=DR,
                    )
            # exp lands on the diagonal slot of the pair's block-diag tile;
            # expsum via DVE reduce of that slot (no ACT accumulator read).
            j = b % 2
            at2 = at2_tiles[(b // 2) % 2]
            nc.scalar.activation(out=at2[:, j, :, j], in_=et, func=AF.Exp)
            nc.vector.tensor_reduce(
                out=expsum_all[:, b : b + 1], in_=at2[:, j, :, j],
                axis=AX.X, op=ALU.add,
            )
            return at2

        def stage_ctx_pair(q, at2, x_nat):
            # Paired ctx: block-diagonal at2 on the two k-tiles against the
            # pair-interleaved x tile -> out[m, e] = row (2q+m)'s ctx partial.
            cps = ctx_ps.tile([2, U], F32, tag="ctxps", name=f"cps{q}")
            for tcc in range(tc_n):
                nc.tensor.matmul(
                    out=cps,
                    lhsT=at2[:, :, tcc, :],
                    rhs=x_nat[:, tcc, :, :],
                    start=(tcc == 0),
                    stop=(tcc == tc_n - 1),
                    perf_mode=DR,
                )
            # Stage straight into ctxn columns via tiny PE transposes (bf16),
            # unnormalized; 1/s is applied once per half-batch.
            stg = small_p.tile([2, U], BF, tag="ctxstg", name=f"stg{q}")
            nc.vector.tensor_copy(stg, cps)
            for e in range(EC):
                tp = tail_ps.tile([P, 2], BF, tag="tail", name=f"ctxT{q}{e}")
                nc.tensor.transpose(tp, stg[:, e * P : (e + 1) * P], idb_sb[0:2, 0:2])
                nc.vector.tensor_copy(ctxn[:, e, 2 * q : 2 * q + 2], tp)

        # ---- tail: per half-batch of 8 rows (h=0 pipelined into the
        # stream at iterations 11-13, h=1 after the loop) ----
        recips_h = [None, None]
        zt_h = [None, None]
        rh_h = [None, None]

        def tail_norm(h):
            rows = slice(HB * h, HB * h + HB)
            s_ps = tail_ps.tile([P, HB], F32, tag="tail", name=f"sps{h}")
            nc.tensor.matmul(out=s_ps, lhsT=ones_sb, rhs=expsum_all[:, rows])
            rec = small_p.tile([P, HB], F32, name=f"recip{h}")
            nc.vector.reciprocal(rec, s_ps)
            recips_h[h] = rec
            for e in range(EC):
                nc.vector.tensor_mul(ctxn[:, e, rows], ctxn[:, e, rows], rec)

        def _gate_psum(h, parts, name):
            # One [P, UC, HB] psum tile accumulating all (weight, rhs) pairs
            # for both u-chunks -> a single ACT covers the whole gate.
            rows = slice(HB * h, HB * h + HB)
            g = tail_ps.tile([P, UC, HB], F32, tag="tail", name=name)
            for uc in range(UC):
                i = 0
                for w_sb, rhs_fn in parts:
                    for e in range(EC):
                        nc.tensor.matmul(
                            out=g[:, uc, :],
                            lhsT=w_sb[:, e, uc * P : (uc + 1) * P],
                            rhs=rhs_fn(e),
                            start=(i == 0),
                            stop=(i == len(parts) * EC - 1),
                        )
                        i += 1
            return g

        def tail_zr(h):
            rows = slice(HB * h, HB * h + HB)
            zt = small_p.tile([P, UC, HB], F32, name=f"zt{h}")
            rh = small_p.tile([P, UC, HB], BF, name=f"rh{h}")
            for gi, wname in ((0, "cz"), (1, "cr")):
                g = _gate_psum(h, [(gate_w[wname], lambda e: ctxn[:, e, rows])], f"g{wname}{h}")
                tmp = small_p.tile([P, UC, HB], F32, tag="gtmp", name=f"t{wname}{h}")
                nc.vector.tensor_add(tmp, g, g0_sb[:, gi, :, rows])
                # sigmoid(v) = 1/(1+exp(-v)): stays in the exp table set.
                ex = small_p.tile([P, UC, HB], F32, tag="gtmp", name=f"e{wname}{h}")
                nc.scalar.activation(out=ex, in_=tmp, func=AF.Exp, scale=-1.0)
                nc.vector.tensor_scalar_add(tmp, ex, 1.0)
                if gi == 0:
                    nc.vector.reciprocal(zt, tmp)
                else:
                    rt = small_p.tile([P, UC, HB], F32, tag="gtmp", name=f"rt{h}")
                    nc.vector.reciprocal(rt, tmp)
                    nc.vector.tensor_mul(rh, rt, hT_sb[:, :, rows])
            zt_h[h], rh_h[h] = zt, rh

        def tail_p_out(h):
            rows = slice(HB * h, HB * h + HB)
            zt, rh = zt_h[h], rh_h[h]
            g = _gate_psum(
                h,
                [(gate_w["up"], lambda e: rh[:, e, :]),
                 (gate_w["cp"], lambda e: ctxn[:, e, rows])],
                f"gp{h}",
            )
            gtmp = small_p.tile([P, UC, HB], F32, tag="gtmp", name=f"gt{h}")
            nc.vector.tensor_add(gtmp, g, g0_sb[:, 2, :, rows])
            tht = small_p.tile([P, UC, HB], F32, tag="gtmp", name=f"tht{h}")
            nc.scalar.activation(out=tht, in_=gtmp, func=AF.Tanh)
            # ht^T = h^T + zt^T*(tht^T - h^T)
            nc.vector.tensor_sub(tht, tht, hT_sb[:, :, rows])
            nc.vector.tensor_mul(tht, tht, zt)
            nc.vector.tensor_add(tht, tht, hT_sb[:, :, rows])
            stg = small_p.tile([HB, U], F32, name=f"htstg{h}")
            for uc in range(UC):
                tp = tail_ps.tile([HB, P], F32, tag="tail", name=f"htp{h}{uc}")
                nc.tensor.transpose(tp, tht[:, uc, :], id_sb)
                nc.vector.tensor_copy(stg[:, uc * P : (uc + 1) * P], tp)
            nc.sync.dma_start(out=ht_d[rows, :], in_=stg)

        # ---- main loop, software-pipelined one row deep ----
        prev = None  # (b, tanh_ts)
        pair_xnat = {}
        for b in range(bs):
            x_nat, xt = stage_dma(b)
            if x_nat is not None:
                pair_xnat[b // 2] = x_nat
            if b == 1:
                load_tail_weights()
            if b == 4:
                load_gate_weights()
            th0 = stage_uxpb_th(b, 0, xt)
            at2_prev = None
            if prev is not None:
                at2_prev = stage_et_exp(prev[0], prev[1])
                pb = prev[0]
            th1 = stage_uxpb_th(b, 1, xt)
            if prev is not None and pb % 2 == 1:
                stage_ctx_pair(pb // 2, at2_prev, pair_xnat.pop(pb // 2))
            if b == 11:
                tail_norm(0)
            elif b == 12:
                tail_zr(0)
            elif b == 13:
                tail_p_out(0)
            prev = (b, [th0, th1])
        at2_last = stage_et_exp(prev[0], prev[1])
        stage_ctx_pair(prev[0] // 2, at2_last, pair_xnat.pop(prev[0] // 2))
        tail_norm(1)
        tail_zr(1)
        tail_p_out(1)

    if split_waits:
        split_multi_waits(nc)
    return nc


def _host_prep(inputs, h_tm, V_a, W_a, U_a, b_a, C_z, W_z, b_z, C_r, W_r, b_r,
               C_p, U_p, b_p):
    """Fold everything not depending on x_seq into small per-core tensors."""
    wxpb = h_tm @ W_a + b_a                                # [B, U]
    g_z0 = h_tm @ W_z + inputs @ C_z[:IN_DIM] + b_z        # [B, U]
    g_r0 = h_tm @ W_r + inputs @ C_r[:IN_DIM] + b_r
    g_p0 = inputs @ C_p[:IN_DIM] + b_p
    shared = {
        "ua": np.ascontiguousarray(U_a.astype(NPF8)),
        "va": np.ascontiguousarray(V_a.reshape(U, 1).astype(NPF8)),
        "cz": np.ascontiguousarray(C_z[IN_DIM:].astype(BF16)),
        "cr": np.ascontiguousarray(C_r[IN_DIM:].astype(BF16)),
        "cp": np.ascontiguousarray(C_p[IN_DIM:].astype(BF16)),
        "up": np.ascontiguousarray(U_p.astype(BF16)),
        "ident": np.eye(P, dtype=np.float32),
    }
    per_core = []
    for c in range(N_CORES):
        s = slice(c * BS, (c + 1) * BS)
        per_core.append(
            {
                "wxpbT": np.ascontiguousarray(wxpb[s].T.astype(np.float32)),
                "hT": np.ascontiguousarray(h_tm[s].T.astype(np.float32)),
                "g0T": np.ascontiguousarray(
                    np.stack([g_z0[s].T, g_r0[s].T, g_p0[s].T]).astype(np.float32)
                ),
                **shared,
            }
        )
    return per_core


def _prep_x(x_core):
    """Pre-tile one core's x [bs, TE, U] into both fp8 layouts."""
    xb = x_core.astype(NPF8)
    tc_n = TE // P
    # xnat[q, p, tc, j, e] = x[2q+j, tc*128+p, e]  (pair-interleaved)
    xnat = np.ascontiguousarray(
        xb.reshape(BS // 2, 2, tc_n, P, U).transpose(0, 3, 2, 1, 4)
    )
    # xtr[b, p, ec, t] = x[b, t, ec*128+p]
    xtr = np.ascontiguousarray(
        xb.reshape(BS, TE, EC, P).transpose(0, 3, 2, 1)
    )
    return xnat, xtr


def build_in_maps(all_inputs):
    """Full host prep: dict of the reference's 16 inputs -> per-core in_maps."""
    args = {k: np.asarray(v, dtype=np.float32) for k, v in all_inputs.items()
            if k != "x_seq"}
    x_seq = np.asarray(all_inputs["x_seq"], dtype=np.float32)
    per_core = _host_prep(**args)
    in_maps = []
    for c in range(N_CORES):
        m = dict(per_core[c])
        m["xnat"], m["xtr"] = _prep_x(x_seq[c * BS : (c + 1) * BS])
        in_maps.append(m)
    return in_maps


def kernel(inputs, h_tm, x_seq, V_a, W_a, U_a, b_a, C_z, W_z, b_z,
           C_r, W_r, b_r, C_p, U_p, b_p):
    from concourse.bass_utils import run_bass_kernel_spmd

    in_maps = build_in_maps(dict(
        inputs=inputs, h_tm=h_tm, x_seq=x_seq, V_a=V_a, W_a=W_a, U_a=U_a,
        b_a=b_a, C_z=C_z, W_z=W_z, b_z=b_z, C_r=C_r, W_r=W_r, b_r=b_r,
        C_p=C_p, U_p=U_p, b_p=b_p))
    nc = build_nc()
    res = run_bass_kernel_spmd(nc, in_maps, core_ids=list(range(N_CORES)))
    return np.concatenate([res.results[c]["ht"] for c in range(N_CORES)], axis=0)


# revision 20
# speedup vs baseline: 1.1121x; 1.1016x over previous
"""AttentionRNNCell Trainium2 kernel (v4).

Math (per batch row b):
  et[t]  = V_a . tanh( (h W_a + b_a) + x[t] U_a )        t in [0, TE)
  at     = exp(et);  s = sum(at)
  ctx    = (sum_t at[t] x[t]) / s
  zt     = sigmoid(h W_z + [inp, ctx] C_z + b_z)
  rt     = sigmoid(h W_r + [inp, ctx] C_r + b_r)
  tht    = tanh((rt*h) U_p + [inp, ctx] C_p + b_p)
  ht     = (1-zt)*h + zt*tht
Distribution: data-parallel over batch B=128 across 8 cores (16 rows each).
Host ships x twice in fp8, pre-tiled in both layouts the PE needs
(xnat: t on partitions for ctx; xtr: e on partitions for uxpb), and folds
everything not depending on x_seq into small per-core tensors.

v4 vs v3 (trace-driven):
  - rows 0/1 xtr go first on the gpsimd (SWDGE) ring in half-row DMAs;
    rows 2-15 ship as PAIR tiles in one DMA each (8KB/partition contiguous
    -> 8KB descriptors; the v3 half-row split produced 1KB descriptors and
    the HWDGE ring drained at only ~76GB/s).
  - xtr pair pool bufs=4 (8 rows of lookahead) so the DMA stream runs at
    ring rate instead of being throttled to compute pace by pool WAR deps
    (v3's row 15 data landed at ~100us, stretching the whole kernel).
  - a dummy ACT right after the preamble pulls the one-time ACT table load
    (~2.7us) off the first-tanh critical path.
  - ctx is transposed + staged per PAIR right after its matmul (DVE copy
    of the [2,256] psum, two tiny PE transposes, DVE copies into ctxn) --
    no SBUF->SBUF ctx_rows DMAs, whose ~2.4us completion latency sat on
    the v3 tail.  Normalization by 1/s happens once per half-batch.
  - gate weights / ctxn / rh are bf16 (fast FWL ldweights; v3's fp32 gate
    LDWEIGHTS were 333ns each) and each gate's two u-chunks share one
    [P,2,8] psum tile -> one ACT per gate (3 per half instead of 6 chains).
  - sigmoid = 1/(1+exp(-x)) on DVE keeps the whole kernel inside the one
    exp_and_others ACT table set.
"""

from contextlib import ExitStack

import numpy as np
import ml_dtypes

import concourse.bass as bass
import concourse.mybir as mybir
import concourse.tile as tile

BF16 = ml_dtypes.bfloat16
NPF8 = ml_dtypes.float8_e4m3
F32 = mybir.dt.float32
BF = mybir.dt.bfloat16
F8 = mybir.dt.float8e4
DR = mybir.MatmulPerfMode.DoubleRow
AF = mybir.ActivationFunctionType
AX = mybir.AxisListType
ALU = mybir.AluOpType

B, TE, U, IN_DIM = 128, 2048, 256, 256
N_CORES = 8
BS = B // N_CORES  # 16 batch rows per core
P = 128
EC = U // P  # e-chunks (2)
UC = U // P  # u-chunks (2)
HB = BS // 2  # rows per tail half (8)


def split_multi_waits(nc, max_waits=1):
    """This container's walrus rejects instructions carrying more than one
    sync wait. Hoist extra waits onto standalone same-engine NoOps inserted
    immediately before the offending instruction (semantically identical:
    the engine blocks on each wait in order before executing it)."""
    n_new = 0
    for f in nc.m.functions:
        for blk in f.blocks:
            new_insts = []
            for inst in blk.instructions:
                si = inst.sync_info
                waits = list(si.on_wait) if si and si.on_wait else []
                if len(waits) > max_waits:
                    for w in waits[:-max_waits]:
                        nop = mybir.InstNoOp(
                            name=f"{inst.name}-hw{n_new}", ins=[], outs=[]
                        )
                        nop.engine = inst.engine
                        nop.sync_info = mybir.SyncInfo(on_wait=[w], on_update=[])
                        new_insts.append(nop)
                        n_new += 1
                    si.on_wait = waits[-max_waits:]
                new_insts.append(inst)
            blk.instructions = new_insts
    return n_new


def build_nc(bs=BS, te=TE, split_waits=True):
    tc_n = te // P      # 128-col t-chunks (16)
    th_n = 2            # t halves
    t_half = te // th_n
    tq_n = t_half // P  # 128-col chunks per half (8)
    n_mm = min(512, t_half)

    nc = bass.Bass()
    xnat_d = nc.declare_dram_parameter("xnat", [bs // 2, P, tc_n, 2, U], F8, isOutput=False)
    xtr_d = nc.declare_dram_parameter("xtr", [bs, P, EC, te], F8, isOutput=False)
    # Small weights ship pre-permuted and packed so every DMA moves >=512
    # contiguous bytes per partition (tiny strided descriptors -- e.g. va as
    # 256 one-byte RMW descriptors -- took >20us on the HWDGE ring and sat
    # in front of the first tanh's bias).
    uav_d = nc.declare_dram_parameter("uav", [P, EC, U + 16], F8, isOutput=False)
    fsm_d = nc.declare_dram_parameter("fsm", [P, 10, bs], F32, isOutput=False)
    cz_d = nc.declare_dram_parameter("cz", [U, U], BF, isOutput=False)
    cr_d = nc.declare_dram_parameter("cr", [U, U], BF, isOutput=False)
    cp_d = nc.declare_dram_parameter("cp", [U, U], BF, isOutput=False)
    up_d = nc.declare_dram_parameter("up", [U, U], BF, isOutput=False)
    id_d = nc.declare_dram_parameter("ident", [P, P], F32, isOutput=False)
    ht_d = nc.declare_dram_parameter("ht", [bs, U], F32, isOutput=True)

    with tile.TileContext(nc) as tc, ExitStack() as ctx:
        singles = ctx.enter_context(tc.tile_pool(name="singles", bufs=1))
        xnat_p = ctx.enter_context(tc.tile_pool(name="xnat", bufs=4))
        xtr_p = ctx.enter_context(tc.tile_pool(name="xtr", bufs=4))
        tanh_p = ctx.enter_context(tc.tile_pool(name="tanh", bufs=8))
        small_p = ctx.enter_context(tc.tile_pool(name="small", bufs=4))
        uxpb_ps = ctx.enter_context(tc.tile_pool(name="uxpbps", bufs=2, space="PSUM"))
        et_ps = ctx.enter_context(tc.tile_pool(name="etps", bufs=1, space="PSUM"))
        ctx_ps = ctx.enter_context(tc.tile_pool(name="ctxps", bufs=1, space="PSUM"))
        tail_ps = ctx.enter_context(tc.tile_pool(name="tailps", bufs=2, space="PSUM"))

        # ---- weights / small per-core tensors ----
        uav_sb = singles.tile([P, EC, U + 16], F8)  # ua cols 0..255, va col 256
        # (k-tile stride padded to 272 = 17*16: dual-fp8 ldweights needs %16==0)
        fsm_sb = singles.tile([P, 10, bs], F32)     # wxpb 0:2, hT 2:4, g0 4:10
        ua_sb = uav_sb
        va_sb = uav_sb[:, :, U : U + 1]

        def load_first_weights():
            # Head of the sync HWDGE ring: ~150KB, lands ~1us after flow
            # starts, ahead of row 0's x.
            nc.sync.dma_start(out=uav_sb, in_=uav_d[:, :, :])
            nc.sync.dma_start(out=fsm_sb, in_=fsm_d[:, :, :])

        gate_w = {}
        for name in ("cz", "cr", "cp", "up"):
            gate_w[name] = singles.tile([P, EC, U], BF, name=f"{name}_sb")
        id_sb = singles.tile([P, P], F32)
        idb_sb = singles.tile([P, P], BF)
        ones_sb = singles.tile([P, P], F32)
        nc.vector.memset(ones_sb, 1.0)
        ones8 = singles.tile([P, 512], F8)
        nc.vector.memset(ones8, 1.0)

        def load_tail_weights():
            # gpsimd ring, after xnat pair 0: needed from ~iteration 2's ctx.
            nc.gpsimd.dma_start(out=id_sb, in_=id_d[:, :])
            nc.vector.tensor_copy(idb_sb, id_sb)

        def load_gate_weights():
            # gpsimd ring, after xnat pair 2 (bf16, 128KB each).
            for name, d in (("cz", cz_d), ("cr", cr_d), ("cp", cp_d), ("up", up_d)):
                nc.gpsimd.dma_start(out=gate_w[name], in_=d[:, :].rearrange("(c p) u -> p c u", p=P))

        expsum_all = singles.tile([P, bs], F32)
        # Unnormalized ctx^T columns, staged per pair via tiny PE transposes
        # (no SBUF->SBUF DMA). Normalized in-place per half-batch.
        ctxn = singles.tile([P, EC, bs], BF)
        # Block-diagonal at tiles for the paired-ctx DoubleRow: slot [j, m]
        # holds row (2q+j)'s at iff j == m, else stays the zero written once
        # here. Two tiles ping-pong across pairs. [p, j, tc, m] layout: the
        # k-tile (j) stride is tc_n*2 bytes (dual-fp8 ldweights needs >=16B).
        at2_tiles = []
        for i in range(2):
            at2 = singles.tile([P, 2, tc_n, 2], F8, name=f"at2_{i}")
            nc.vector.memset(at2, 0.0)
            at2_tiles.append(at2)

        # ---- ACT table preload + HAM warmup, both during the initial DMA
        # wait: the weight DMAs issue first on the scalar ring, then a dummy
        # ACT pulls the one-time exp_and_others table load off the
        # first-tanh critical path; ~2.6us of dummy matmuls get the PE
        # clock-gate warming before row 0's data lands.
        load_first_weights()
        actwarm = small_p.tile([P, 1], F32, name="actwarm")
        nc.scalar.activation(out=actwarm, in_=ones_sb[:, 0:1], func=AF.Tanh)
        warm = tail_ps.tile([P, 512], F32, tag="tail", name="warm")
        for _ in range(6):
            nc.tensor.matmul(out=warm, lhsT=ones8[:, 0:P], rhs=ones8)

        # ---- streaming stages ----
        pend_pair = {}

        def stage_dma(b):
            x_nat = None
            if b % 2 == 0:
                x_nat = xnat_p.tile([P, tc_n, 2, U], F8, tag="xnat", name=f"xnat{b}")
                # pair 7's xnat fills the sync ring's tail; the rest ride
                # gpsimd (SWDGE has a ~7us cold-start, so nothing
                # startup-critical goes there).
                eng = nc.sync if b == 14 else nc.gpsimd
                eng.dma_start(out=x_nat, in_=xnat_d[b // 2])
            if b < 2:
                # Rows 0/1: one full-row DMA each, right behind the packed
                # weights on the sync ring (4KB/partition contiguous).
                xt = singles.tile([P, EC, te], F8, name=f"xt{b}")
                nc.sync.dma_start(out=xt, in_=xtr_d[b])
            elif b % 2 == 0:
                # One DMA per pair: per-partition-contiguous 2x4KB source
                # blocks -> big descriptors, full HWDGE ring rate.
                xp = xtr_p.tile([P, 2, EC, te], F8, tag="xt", name=f"xt{b}")
                nc.sync.dma_start(
                    out=xp, in_=xtr_d[b : b + 2].rearrange("j p c t -> p j c t")
                )
                pend_pair[b // 2] = xp
                xt = xp[:, 0]
            else:
                xt = pend_pair.pop(b // 2)[:, 1]
            return x_nat, xt

        def stage_uxpb_th(b, th, xt):
            # uxpb: out[u, t] = sum_e ua[e, u] * xt[e, t] -- fp8 DoubleRow
            # contracts both e-chunks in one matmul. tanh (per-partition
            # bias) -> SBUF fp8 [u, uc, t] tiles for the et DoubleRow.
            tanh_t = tanh_p.tile([P, UC, t_half], F8, tag="tanh", name=f"th{b}_{th}")
            for uc in range(UC):
                ux = uxpb_ps.tile([P, t_half], F32, tag="ux", name=f"ux{b}{th}{uc}")
                for n0 in range(0, t_half, n_mm):
                    nc.tensor.matmul(
                        out=ux[:, n0 : n0 + n_mm],
                        lhsT=ua_sb[:, :, uc * P : (uc + 1) * P],
                        rhs=xt[:, :, th * t_half + n0 : th * t_half + n0 + n_mm],
                        perf_mode=DR,
                    )
                nc.scalar.activation(
                    out=tanh_t[:, uc, :], in_=ux, func=AF.Tanh,
                    bias=fsm_sb[:, uc, b : b + 1],
                )
            return tanh_t

        def stage_et_exp(b, tanh_ts):
            et = et_ps.tile([P, tc_n], F32, tag="etp", name=f"et{b}")
            for th in range(th_n):
                for tq in range(tq_n):
                    nc.tensor.matmul(
                        out=et[:, th * tq_n + tq : th * tq_n + tq + 1],
                        lhsT=tanh_ts[th][:, :, tq * P : (tq + 1) * P],
                        rhs=va_sb,
                        perf_mode=DR,
                    )
            # exp lands on the diagonal slot of the pair's block-diag tile;
            # expsum via DVE reduce of that slot (no ACT accumulator read).
            j = b % 2
            at2 = at2_tiles[(b // 2) % 2]
            nc.scalar.activation(out=at2[:, j, :, j], in_=et, func=AF.Exp)
            nc.vector.tensor_reduce(
                out=expsum_all[:, b : b + 1], in_=at2[:, j, :, j],
                axis=AX.X, op=ALU.add,
            )
            return at2

        def stage_ctx_pair(q, at2, x_nat):
            # Paired ctx: block-diagonal at2 on the two k-tiles against the
            # pair-interleaved x tile -> out[m, e] = row (2q+m)'s ctx partial.
            cps = ctx_ps.tile([2, U], F32, tag="ctxps", name=f"cps{q}")
            for tcc in range(tc_n):
                nc.tensor.matmul(
                    out=cps,
                    lhsT=at2[:, :, tcc, :],
                    rhs=x_nat[:, tcc, :, :],
                    start=(tcc == 0),
                    stop=(tcc == tc_n - 1),
                    perf_mode=DR,
                )
            # Stage straight into ctxn columns via tiny PE transposes (bf16),
            # unnormalized; 1/s is applied once per half-batch.
            stg = small_p.tile([2, U], BF, tag="ctxstg", name=f"stg{q}")
            nc.vector.tensor_copy(stg, cps)
            for e in range(EC):
                tp = tail_ps.tile([P, 2], BF, tag="tail", name=f"ctxT{q}{e}")
                nc.tensor.transpose(tp, stg[:, e * P : (e + 1) * P], idb_sb[0:2, 0:2])
                nc.vector.tensor_copy(ctxn[:, e, 2 * q : 2 * q + 2], tp)

        # ---- tail: per half-batch of 8 rows (h=0 pipelined into the
        # stream at iterations 11-13, h=1 after the loop) ----
        recips_h = [None, None]
        zt_h = [None, None]
        rh_h = [None, None]

        def tail_norm(h):
            rows = slice(HB * h, HB * h + HB)
            s_ps = tail_ps.tile([P, HB], F32, tag="tail", name=f"sps{h}")
            nc.tensor.matmul(out=s_ps, lhsT=ones_sb, rhs=expsum_all[:, rows])
            rec = small_p.tile([P, HB], F32, name=f"recip{h}")
            nc.vector.reciprocal(rec, s_ps)
            recips_h[h] = rec
            for e in range(EC):
                nc.vector.tensor_mul(ctxn[:, e, rows], ctxn[:, e, rows], rec)

        def _gate_psum(h, parts, name):
            # One [P, UC, HB] psum tile accumulating all (weight, rhs) pairs
            # for both u-chunks -> a single ACT covers the whole gate.
            rows = slice(HB * h, HB * h + HB)
            g = tail_ps.tile([P, UC, HB], F32, tag="tail", name=name)
            for uc in range(UC):
                i = 0
                for w_sb, rhs_fn in parts:
                    for e in range(EC):
                        nc.tensor.matmul(
                            out=g[:, uc, :],
                            lhsT=w_sb[:, e, uc * P : (uc + 1) * P],
                            rhs=rhs_fn(e),
                            start=(i == 0),
                            stop=(i == len(parts) * EC - 1),
                        )
                        i += 1
            return g

        def tail_zr(h):
            rows = slice(HB * h, HB * h + HB)
            zt = small_p.tile([P, UC, HB], F32, name=f"zt{h}")
            rh = small_p.tile([P, UC, HB], BF, name=f"rh{h}")
            for gi, wname in ((0, "cz"), (1, "cr")):
                g = _gate_psum(h, [(gate_w[wname], lambda e: ctxn[:, e, rows])], f"g{wname}{h}")
                tmp = small_p.tile([P, UC, HB], F32, tag="gtmp", name=f"t{wname}{h}")
                nc.vector.tensor_add(tmp, g, fsm_sb[:, 4 + 2 * gi : 6 + 2 * gi, rows])
                # sigmoid(v) = 1/(1+exp(-v)): stays in the exp table set.
                ex = small_p.tile([P, UC, HB], F32, tag="gtmp", name=f"e{wname}{h}")
                nc.scalar.activation(out=ex, in_=tmp, func=AF.Exp, scale=-1.0)
                nc.vector.tensor_scalar_add(tmp, ex, 1.0)
                if gi == 0:
                    nc.vector.reciprocal(zt, tmp)
                else:
                    rt = small_p.tile([P, UC, HB], F32, tag="gtmp", name=f"rt{h}")
                    nc.vector.reciprocal(rt, tmp)
                    nc.vector.tensor_mul(rh, rt, fsm_sb[:, 2:4, rows])
            zt_h[h], rh_h[h] = zt, rh

        def tail_p_out(h):
            rows = slice(HB * h, HB * h + HB)
            zt, rh = zt_h[h], rh_h[h]
            g = _gate_psum(
                h,
                [(gate_w["up"], lambda e: rh[:, e, :]),
                 (gate_w["cp"], lambda e: ctxn[:, e, rows])],
                f"gp{h}",
            )
            gtmp = small_p.tile([P, UC, HB], F32, tag="gtmp", name=f"gt{h}")
            nc.vector.tensor_add(gtmp, g, fsm_sb[:, 8:10, rows])
            tht = small_p.tile([P, UC, HB], F32, tag="gtmp", name=f"tht{h}")
            nc.scalar.activation(out=tht, in_=gtmp, func=AF.Tanh)
            # ht^T = h^T + zt^T*(tht^T - h^T)
            nc.vector.tensor_sub(tht, tht, fsm_sb[:, 2:4, rows])
            nc.vector.tensor_mul(tht, tht, zt)
            nc.vector.tensor_add(tht, tht, fsm_sb[:, 2:4, rows])
            stg = small_p.tile([HB, U], F32, name=f"htstg{h}")
            for uc in range(UC):
                tp = tail_ps.tile([HB, P], F32, tag="tail", name=f"htp{h}{uc}")
                nc.tensor.transpose(tp, tht[:, uc, :], id_sb)
                nc.vector.tensor_copy(stg[:, uc * P : (uc + 1) * P], tp)
            nc.sync.dma_start(out=ht_d[rows, :], in_=stg)

        # ---- main loop, software-pipelined one row deep ----
        prev = None  # (b, tanh_ts)
        pair_xnat = {}
        for b in range(bs):
            x_nat, xt = stage_dma(b)
            if x_nat is not None:
                pair_xnat[b // 2] = x_nat
            if b == 1:
                load_tail_weights()
            if b == 4:
                load_gate_weights()
            th0 = stage_uxpb_th(b, 0, xt)
            at2_prev = None
            if prev is not None:
                at2_prev = stage_et_exp(prev[0], prev[1])
                pb = prev[0]
            th1 = stage_uxpb_th(b, 1, xt)
            if prev is not None and pb % 2 == 1:
                stage_ctx_pair(pb // 2, at2_prev, pair_xnat.pop(pb // 2))
            if b == 11:
                tail_norm(0)
            elif b == 12:
                tail_zr(0)
            elif b == 13:
                tail_p_out(0)
            prev = (b, [th0, th1])
        at2_last = stage_et_exp(prev[0], prev[1])
        stage_ctx_pair(prev[0] // 2, at2_last, pair_xnat.pop(prev[0] // 2))
        tail_norm(1)
        tail_zr(1)
        tail_p_out(1)

    if split_waits:
        split_multi_waits(nc)
    return nc


def _host_prep(inputs, h_tm, V_a, W_a, U_a, b_a, C_z, W_z, b_z, C_r, W_r, b_r,
               C_p, U_p, b_p):
    """Fold everything not depending on x_seq into small per-core tensors."""
    wxpb = h_tm @ W_a + b_a                                # [B, U]
    g_z0 = h_tm @ W_z + inputs @ C_z[:IN_DIM] + b_z        # [B, U]
    g_r0 = h_tm @ W_r + inputs @ C_r[:IN_DIM] + b_r
    g_p0 = inputs @ C_p[:IN_DIM] + b_p
    # uav[p, c, :]: ua row (c*128+p) in cols 0..U-1, va[c*128+p] in col U --
    # one packed DMA with >=512 contiguous bytes per partition.
    uav = np.zeros((P, EC, U + 16), dtype=NPF8)
    uav[:, :, :U] = U_a.astype(NPF8).reshape(EC, P, U).transpose(1, 0, 2)
    uav[:, :, U] = V_a.astype(NPF8).reshape(EC, P).T
    shared = {
        "uav": np.ascontiguousarray(uav),
        "cz": np.ascontiguousarray(C_z[IN_DIM:].astype(BF16)),
        "cr": np.ascontiguousarray(C_r[IN_DIM:].astype(BF16)),
        "cp": np.ascontiguousarray(C_p[IN_DIM:].astype(BF16)),
        "up": np.ascontiguousarray(U_p.astype(BF16)),
        "ident": np.eye(P, dtype=np.float32),
    }
    per_core = []
    for c in range(N_CORES):
        s = slice(c * BS, (c + 1) * BS)
        # fsm[p, 0:2]: wxpb^T; [p, 2:4]: h^T; [p, 4:10]: g0^T for z, r, p --
        # all [u%128 -> p, u//128 -> chunk, b] layouts packed in one tensor.
        def chunked(m):  # [bs, U] -> [P, UC, bs]
            return m.T.astype(np.float32).reshape(UC, P, BS).transpose(1, 0, 2)
        fsm = np.concatenate(
            [chunked(wxpb[s]), chunked(h_tm[s]), chunked(g_z0[s]),
             chunked(g_r0[s]), chunked(g_p0[s])], axis=1
        )
        per_core.append({"fsm": np.ascontiguousarray(fsm), **shared})
    return per_core


def _prep_x(x_core):
    """Pre-tile one core's x [bs, TE, U] into both fp8 layouts."""
    xb = x_core.astype(NPF8)
    tc_n = TE // P
    # xnat[q, p, tc, j, e] = x[2q+j, tc*128+p, e]  (pair-interleaved)
    xnat = np.ascontiguousarray(
        xb.reshape(BS // 2, 2, tc_n, P, U).transpose(0, 3, 2, 1, 4)
    )
    # xtr[b, p, ec, t] = x[b, t, ec*128+p]
    xtr = np.ascontiguousarray(
        xb.reshape(BS, TE, EC, P).transpose(0, 3, 2, 1)
    )
    return xnat, xtr


def build_in_maps(all_inputs):
    """Full host prep: dict of the reference's 16 inputs -> per-core in_maps."""
    args = {k: np.asarray(v, dtype=np.float32) for k, v in all_inputs.items()
            if k != "x_seq"}
    x_seq = np.asarray(all_inputs["x_seq"], dtype=np.float32)
    per_core = _host_prep(**args)
    in_maps = []
    for c in range(N_CORES):
        m = dict(per_core[c])
        m["xnat"], m["xtr"] = _prep_x(x_seq[c * BS : (c + 1) * BS])
        in_maps.append(m)
    return in_maps


def kernel(inputs, h_tm, x_seq, V_a, W_a, U_a, b_a, C_z, W_z, b_z,
           C_r, W_r, b_r, C_p, U_p, b_p):
    from concourse.bass_utils import run_bass_kernel_spmd

    in_maps = build_in_maps(dict(
        inputs=inputs, h_tm=h_tm, x_seq=x_seq, V_a=V_a, W_a=W_a, U_a=U_a,
        b_a=b_a, C_z=C_z, W_z=W_z, b_z=b_z, C_r=C_r, W_r=W_r, b_r=b_r,
        C_p=C_p, U_p=U_p, b_p=b_p))
    nc = build_nc()
    res = run_bass_kernel_spmd(nc, in_maps, core_ids=list(range(N_CORES)))
    return np.concatenate([res.results[c]["ht"] for c in range(N_CORES)], axis=0)


# revision 23
# speedup vs baseline: 1.1562x; 1.0396x over previous
"""AttentionRNNCell Trainium2 kernel (v4).

Math (per batch row b):
  et[t]  = V_a . tanh( (h W_a + b_a) + x[t] U_a )        t in [0, TE)
  at     = exp(et);  s = sum(at)
  ctx    = (sum_t at[t] x[t]) / s
  zt     = sigmoid(h W_z + [inp, ctx] C_z + b_z)
  rt     = sigmoid(h W_r + [inp, ctx] C_r + b_r)
  tht    = tanh((rt*h) U_p + [inp, ctx] C_p + b_p)
  ht     = (1-zt)*h + zt*tht
Distribution: data-parallel over batch B=128 across 8 cores (16 rows each).
Host ships x twice in fp8, pre-tiled in both layouts the PE needs
(xnat: t on partitions for ctx; xtr: e on partitions for uxpb), and folds
everything not depending on x_seq into small per-core tensors.

v4 vs v3 (trace-driven):
  - rows 0/1 xtr go first on the gpsimd (SWDGE) ring in half-row DMAs;
    rows 2-15 ship as PAIR tiles in one DMA each (8KB/partition contiguous
    -> 8KB descriptors; the v3 half-row split produced 1KB descriptors and
    the HWDGE ring drained at only ~76GB/s).
  - xtr pair pool bufs=4 (8 rows of lookahead) so the DMA stream runs at
    ring rate instead of being throttled to compute pace by pool WAR deps
    (v3's row 15 data landed at ~100us, stretching the whole kernel).
  - a dummy ACT right after the preamble pulls the one-time ACT table load
    (~2.7us) off the first-tanh critical path.
  - ctx is transposed + staged per PAIR right after its matmul (DVE copy
    of the [2,256] psum, two tiny PE transposes, DVE copies into ctxn) --
    no SBUF->SBUF ctx_rows DMAs, whose ~2.4us completion latency sat on
    the v3 tail.  Normalization by 1/s happens once per half-batch.
  - gate weights / ctxn / rh are bf16 (fast FWL ldweights; v3's fp32 gate
    LDWEIGHTS were 333ns each) and each gate's two u-chunks share one
    [P,2,8] psum tile -> one ACT per gate (3 per half instead of 6 chains).
  - sigmoid = 1/(1+exp(-x)) on DVE keeps the whole kernel inside the one
    exp_and_others ACT table set.
"""

from contextlib import ExitStack

import numpy as np
import ml_dtypes

import concourse.bass as bass
import concourse.mybir as mybir
import concourse.tile as tile

BF16 = ml_dtypes.bfloat16
NPF8 = ml_dtypes.float8_e4m3
F32 = mybir.dt.float32
BF = mybir.dt.bfloat16
F8 = mybir.dt.float8e4
DR = mybir.MatmulPerfMode.DoubleRow
AF = mybir.ActivationFunctionType
AX = mybir.AxisListType
ALU = mybir.AluOpType

B, TE, U, IN_DIM = 128, 2048, 256, 256
N_CORES = 8
BS = B // N_CORES  # 16 batch rows per core
P = 128
EC = U // P  # e-chunks (2)
UC = U // P  # u-chunks (2)
HB = BS // 2  # rows per tail half (8)


def split_multi_waits(nc, max_waits=1):
    """This container's walrus rejects instructions carrying more than one
    sync wait. Hoist extra waits onto standalone same-engine NoOps inserted
    immediately before the offending instruction (semantically identical:
    the engine blocks on each wait in order before executing it)."""
    n_new = 0
    for f in nc.m.functions:
        for blk in f.blocks:
            new_insts = []
            for inst in blk.instructions:
                si = inst.sync_info
                waits = list(si.on_wait) if si and si.on_wait else []
                if len(waits) > max_waits:
                    for w in waits[:-max_waits]:
                        nop = mybir.InstNoOp(
                            name=f"{inst.name}-hw{n_new}", ins=[], outs=[]
                        )
                        nop.engine = inst.engine
                        nop.sync_info = mybir.SyncInfo(on_wait=[w], on_update=[])
                        new_insts.append(nop)
                        n_new += 1
                    si.on_wait = waits[-max_waits:]
                new_insts.append(inst)
            blk.instructions = new_insts
    return n_new


def build_nc(bs=BS, te=TE, split_waits=True):
    tc_n = te // P      # 128-col t-chunks (16)
    th_n = 2            # t halves
    t_half = te // th_n
    tq_n = t_half // P  # 128-col chunks per half (8)
    n_mm = min(512, t_half)

    nc = bass.Bass()
    xnat_d = nc.declare_dram_parameter("xnat", [bs // 2, P, tc_n, 2, U], F8, isOutput=False)
    xtr_d = nc.declare_dram_parameter("xtr", [bs, P, EC, te], F8, isOutput=False)
    # Small weights ship pre-permuted and packed so every DMA moves >=512
    # contiguous bytes per partition (tiny strided descriptors -- e.g. va as
    # 256 one-byte RMW descriptors -- took >20us on the HWDGE ring and sat
    # in front of the first tanh's bias).
    uav_d = nc.declare_dram_parameter("uav", [P, EC, U + 16], F8, isOutput=False)
    fsm_d = nc.declare_dram_parameter("fsm", [P, 10, bs], F32, isOutput=False)
    cz_d = nc.declare_dram_parameter("cz", [U, U], BF, isOutput=False)
    cr_d = nc.declare_dram_parameter("cr", [U, U], BF, isOutput=False)
    cp_d = nc.declare_dram_parameter("cp", [U, U], BF, isOutput=False)
    up_d = nc.declare_dram_parameter("up", [U, U], BF, isOutput=False)
    id_d = nc.declare_dram_parameter("ident", [P, P], F32, isOutput=False)
    ht_d = nc.declare_dram_parameter("ht", [bs, U], F32, isOutput=True)

    with tile.TileContext(nc) as tc, ExitStack() as ctx:
        singles = ctx.enter_context(tc.tile_pool(name="singles", bufs=1))
        xnat_p = ctx.enter_context(tc.tile_pool(name="xnat", bufs=3))
        xtr_p = ctx.enter_context(tc.tile_pool(name="xtr", bufs=4))
        tanh_p = ctx.enter_context(tc.tile_pool(name="tanh", bufs=8))
        small_p = ctx.enter_context(tc.tile_pool(name="small", bufs=4))
        uxpb_ps = ctx.enter_context(tc.tile_pool(name="uxpbps", bufs=3, space="PSUM"))
        et_ps = ctx.enter_context(tc.tile_pool(name="etps", bufs=1, space="PSUM"))
        tail_ps = ctx.enter_context(tc.tile_pool(name="tailps", bufs=1, space="PSUM"))

        # ---- weights / small per-core tensors ----
        uav_sb = singles.tile([P, EC, U + 16], F8)  # ua cols 0..255, va col 256
        # (k-tile stride padded to 272 = 17*16: dual-fp8 ldweights needs %16==0)
        fsm_sb = singles.tile([P, 10, bs], F32)     # wxpb 0:2, hT 2:4, g0 4:10
        ua_sb = uav_sb
        va_sb = uav_sb[:, :, U : U + 1]

        def load_first_weights():
            # Head of the sync HWDGE ring: ~150KB, lands ~1us after flow
            # starts, ahead of row 0's x.
            nc.sync.dma_start(out=uav_sb, in_=uav_d[:, :, :])
            nc.sync.dma_start(out=fsm_sb, in_=fsm_d[:, :, :])

        gate_w = {}
        for name in ("cz", "cr", "cp", "up"):
            gate_w[name] = singles.tile([P, EC, U], BF, name=f"{name}_sb")
        id_sb = singles.tile([P, P], F32)
        idb_sb = singles.tile([P, P], BF)
        ones_sb = singles.tile([P, P], F32)
        nc.vector.memset(ones_sb, 1.0)
        ones8 = singles.tile([P, 512], F8)
        nc.vector.memset(ones8, 1.0)

        def load_tail_weights():
            # sync ring, right after row 1: needed from ~iteration 3's ctx.
            nc.sync.dma_start(out=id_sb, in_=id_d[:, :])
            nc.vector.tensor_copy(idb_sb, id_sb)

        def load_gate_weights():
            # sync ring, after pair (6,7)'s xtr (bf16, 128KB each).
            for name, d in (("cz", cz_d), ("cr", cr_d), ("cp", cp_d), ("up", up_d)):
                nc.sync.dma_start(out=gate_w[name], in_=d[:, :].rearrange("(c p) u -> p c u", p=P))

        expsum_all = singles.tile([P, bs], F32)
        # Unnormalized ctx^T columns, staged per pair via tiny PE transposes
        # (no SBUF->SBUF DMA). Normalized in-place per half-batch.
        ctxn = singles.tile([P, EC, bs], BF)
        # Block-diagonal at tiles for the paired-ctx DoubleRow: slot [j, m]
        # holds row (2q+j)'s at iff j == m, else stays the zero written once
        # here. Two tiles ping-pong across pairs. [p, j, tc, m] layout: the
        # k-tile (j) stride is tc_n*2 bytes (dual-fp8 ldweights needs >=16B).
        at2_tiles = []
        for i in range(2):
            at2 = singles.tile([P, 2, tc_n, 2], F8, name=f"at2_{i}")
            nc.vector.memset(at2, 0.0)
            at2_tiles.append(at2)

        # ---- ACT table preload + HAM warmup, both during the initial DMA
        # wait: the weight DMAs issue first on the scalar ring, then a dummy
        # ACT pulls the one-time exp_and_others table load off the
        # first-tanh critical path; ~2.6us of dummy matmuls get the PE
        # clock-gate warming before row 0's data lands.
        load_first_weights()
        actwarm = small_p.tile([P, 1], F32, name="actwarm")
        nc.scalar.activation(out=actwarm, in_=ones_sb[:, 0:1], func=AF.Tanh)
        warm = uxpb_ps.tile([P, 512], F32, tag="ux", name="warm")
        for _ in range(6):
            nc.tensor.matmul(out=warm, lhsT=ones8[:, 0:P], rhs=ones8)

        # ---- streaming stages ----
        pend_pair = {}

        def stage_dma(b):
            x_nat = None
            if b % 2 == 0:
                x_nat = xnat_p.tile([P, tc_n, 2, U], F8, tag="xnat", name=f"xnat{b}")
                # Pairs 0/1 ride the sync ring: the SDMA engines round-robin
                # between rings at packet granularity, and SWDGE's 8KB
                # descriptors would starve the startup-critical small
                # transfers.  From pair 2 on, the xnat pool's WAR dep (bufs=3)
                # holds the SWDGE ring back until pair-0's ctx is done.
                eng = nc.sync if b < 4 else nc.gpsimd
                eng.dma_start(out=x_nat, in_=xnat_d[b // 2])
            if b < 2:
                # Rows 0/1: one full-row DMA each, right behind the packed
                # weights on the sync ring (4KB/partition contiguous).
                xt = singles.tile([P, EC, te], F8, name=f"xt{b}")
                nc.sync.dma_start(out=xt, in_=xtr_d[b])
            elif b % 2 == 0:
                # One DMA per pair: per-partition-contiguous 2x4KB source
                # blocks -> big descriptors, full HWDGE ring rate.
                xp = xtr_p.tile([P, 2, EC, te], F8, tag="xt", name=f"xt{b}")
                nc.sync.dma_start(
                    out=xp, in_=xtr_d[b : b + 2].rearrange("j p c t -> p j c t")
                )
                pend_pair[b // 2] = xp
                xt = xp[:, 0]
            else:
                xt = pend_pair.pop(b // 2)[:, 1]
            return x_nat, xt

        def stage_uxpb_th(b, th, xt):
            # uxpb: out[u, t] = sum_e ua[e, u] * xt[e, t] -- fp8 DoubleRow
            # contracts both e-chunks in one matmul. tanh (per-partition
            # bias) -> SBUF fp8 [u, uc, t] tiles for the et DoubleRow.
            tanh_t = tanh_p.tile([P, UC, t_half], F8, tag="tanh", name=f"th{b}_{th}")
            for uc in range(UC):
                ux = uxpb_ps.tile([P, t_half], F32, tag="ux", name=f"ux{b}{th}{uc}")
                for n0 in range(0, t_half, n_mm):
                    nc.tensor.matmul(
                        out=ux[:, n0 : n0 + n_mm],
                        lhsT=ua_sb[:, :, uc * P : (uc + 1) * P],
                        rhs=xt[:, :, th * t_half + n0 : th * t_half + n0 + n_mm],
                        perf_mode=DR,
                    )
                nc.scalar.activation(
                    out=tanh_t[:, uc, :], in_=ux, func=AF.Tanh,
                    bias=fsm_sb[:, uc, b : b + 1],
                )
            return tanh_t

        def stage_et_exp(b, tanh_ts):
            et = et_ps.tile([P, tc_n], F32, tag="etp", name=f"et{b}")
            for th in range(th_n):
                for tq in range(tq_n):
                    nc.tensor.matmul(
                        out=et[:, th * tq_n + tq : th * tq_n + tq + 1],
                        lhsT=tanh_ts[th][:, :, tq * P : (tq + 1) * P],
                        rhs=va_sb,
                        perf_mode=DR,
                    )
            # exp lands on the diagonal slot of the pair's block-diag tile;
            # expsum via DVE reduce of that slot (no ACT accumulator read).
            j = b % 2
            at2 = at2_tiles[(b // 2) % 2]
            nc.scalar.activation(out=at2[:, j, :, j], in_=et, func=AF.Exp)
            nc.vector.tensor_reduce(
                out=expsum_all[:, b : b + 1], in_=at2[:, j, :, j],
                axis=AX.X, op=ALU.add,
            )
            return at2

        def stage_ctx_half(q, at2, x_nat, cps, lo, hi):
            # Paired ctx: block-diagonal at2 on the two k-tiles against the
            # pair-interleaved x tile -> out[m, e] = row (2q+m)'s ctx partial.
            # Issued as two 8-matmul chunks in consecutive iterations so PE
            # load stays balanced against the ACT tanh pace.
            if cps is None:
                cps = tail_ps.tile([2, U], F32, tag="tail", name=f"cps{q}")
            for tcc in range(lo, hi):
                nc.tensor.matmul(
                    out=cps,
                    lhsT=at2[:, :, tcc, :],
                    rhs=x_nat[:, tcc, :, :],
                    start=(tcc == 0),
                    stop=(tcc == tc_n - 1),
                    perf_mode=DR,
                )
            return cps

        def stage_ctx_finish(q, cps):
            # Stage straight into ctxn columns via tiny PE transposes (bf16),
            # unnormalized; 1/s is applied once per half-batch.
            stg = small_p.tile([2, U], BF, tag="ctxstg", name=f"stg{q}")
            nc.vector.tensor_copy(stg, cps)
            for e in range(EC):
                tp = tail_ps.tile([P, 2], BF, tag="tail", name=f"ctxT{q}{e}")
                nc.tensor.transpose(tp, stg[:, e * P : (e + 1) * P], idb_sb[0:2, 0:2])
                nc.vector.tensor_copy(ctxn[:, e, 2 * q : 2 * q + 2], tp)

        # ---- tail: per half-batch of 8 rows (h=0 pipelined into the
        # stream at iterations 11-13, h=1 after the loop) ----
        recips_h = [None, None]
        zt_h = [None, None]
        rh_h = [None, None]

        def tail_norm(h):
            rows = slice(HB * h, HB * h + HB)
            s_ps = tail_ps.tile([P, HB], F32, tag="tail", name=f"sps{h}")
            nc.tensor.matmul(out=s_ps, lhsT=ones_sb, rhs=expsum_all[:, rows])
            rec = small_p.tile([P, HB], F32, name=f"recip{h}")
            nc.vector.reciprocal(rec, s_ps)
            recips_h[h] = rec
            for e in range(EC):
                nc.vector.tensor_mul(ctxn[:, e, rows], ctxn[:, e, rows], rec)

        def _gate_psum(h, parts, name):
            # One [P, UC, HB] psum tile accumulating all (weight, rhs) pairs
            # for both u-chunks -> a single ACT covers the whole gate.
            rows = slice(HB * h, HB * h + HB)
            g = tail_ps.tile([P, UC, HB], F32, tag="tail", name=name)
            for uc in range(UC):
                i = 0
                for w_sb, rhs_fn in parts:
                    for e in range(EC):
                        nc.tensor.matmul(
                            out=g[:, uc, :],
                            lhsT=w_sb[:, e, uc * P : (uc + 1) * P],
                            rhs=rhs_fn(e),
                            start=(i == 0),
                            stop=(i == len(parts) * EC - 1),
                        )
                        i += 1
            return g

        def tail_zr(h):
            rows = slice(HB * h, HB * h + HB)
            zt = small_p.tile([P, UC, HB], F32, name=f"zt{h}")
            rh = small_p.tile([P, UC, HB], BF, name=f"rh{h}")
            for gi, wname in ((0, "cz"), (1, "cr")):
                g = _gate_psum(h, [(gate_w[wname], lambda e: ctxn[:, e, rows])], f"g{wname}{h}")
                tmp = small_p.tile([P, UC, HB], F32, tag="gtmp", name=f"t{wname}{h}")
                nc.vector.tensor_add(tmp, g, fsm_sb[:, 4 + 2 * gi : 6 + 2 * gi, rows])
                # sigmoid(v) = 1/(1+exp(-v)): stays in the exp table set.
                ex = small_p.tile([P, UC, HB], F32, tag="gtmp", name=f"e{wname}{h}")
                nc.scalar.activation(out=ex, in_=tmp, func=AF.Exp, scale=-1.0)
                nc.vector.tensor_scalar_add(tmp, ex, 1.0)
                if gi == 0:
                    nc.vector.reciprocal(zt, tmp)
                else:
                    rt = small_p.tile([P, UC, HB], F32, tag="gtmp", name=f"rt{h}")
                    nc.vector.reciprocal(rt, tmp)
                    nc.vector.tensor_mul(rh, rt, fsm_sb[:, 2:4, rows])
            zt_h[h], rh_h[h] = zt, rh

        def tail_p_out(h):
            rows = slice(HB * h, HB * h + HB)
            zt, rh = zt_h[h], rh_h[h]
            g = _gate_psum(
                h,
                [(gate_w["up"], lambda e: rh[:, e, :]),
                 (gate_w["cp"], lambda e: ctxn[:, e, rows])],
                f"gp{h}",
            )
            gtmp = small_p.tile([P, UC, HB], F32, tag="gtmp", name=f"gt{h}")
            nc.vector.tensor_add(gtmp, g, fsm_sb[:, 8:10, rows])
            tht = small_p.tile([P, UC, HB], F32, tag="gtmp", name=f"tht{h}")
            nc.scalar.activation(out=tht, in_=gtmp, func=AF.Tanh)
            # ht^T = h^T + zt^T*(tht^T - h^T)
            nc.vector.tensor_sub(tht, tht, fsm_sb[:, 2:4, rows])
            nc.vector.tensor_mul(tht, tht, zt)
            nc.vector.tensor_add(tht, tht, fsm_sb[:, 2:4, rows])
            stg = small_p.tile([HB, U], F32, name=f"htstg{h}")
            for uc in range(UC):
                tp = tail_ps.tile([HB, P], F32, tag="tail", name=f"htp{h}{uc}")
                nc.tensor.transpose(tp, tht[:, uc, :], id_sb)
                nc.vector.tensor_copy(stg[:, uc * P : (uc + 1) * P], tp)
            nc.sync.dma_start(out=ht_d[rows, :], in_=stg)

        def keepalive(i):
            # One matmul to reset the PE clock-gate's idle window during the
            # latency-bound tail (else it re-throttles to 1.2GHz mid-tail).
            ka = uxpb_ps.tile([P, 512], F32, tag="ux", name=f"ka{i}")
            nc.tensor.matmul(out=ka, lhsT=ones8[:, 0:P], rhs=ones8)

        # ---- main loop, software-pipelined one row deep ----
        prev = None  # (b, tanh_ts)
        pair_xnat = {}
        pend_ctx = None  # (q, cps) with 8 of 16 matmuls issued
        for b in range(bs):
            x_nat, xt = stage_dma(b)
            if x_nat is not None:
                pair_xnat[b // 2] = x_nat
            if b == 1:
                load_tail_weights()
            if b == 6:
                load_gate_weights()
            th0 = stage_uxpb_th(b, 0, xt)
            if pend_ctx is not None:
                q, cps = pend_ctx
                stage_ctx_finish(q, stage_ctx_half(q, at2_tiles[q % 2], pair_xnat.pop(q), cps, tc_n // 2, tc_n))
                pend_ctx = None
            at2_prev = None
            if prev is not None:
                at2_prev = stage_et_exp(prev[0], prev[1])
                pb = prev[0]
            th1 = stage_uxpb_th(b, 1, xt)
            if prev is not None and pb % 2 == 1 and pb < bs - 1:
                q = pb // 2
                pend_ctx = (q, stage_ctx_half(q, at2_prev, pair_xnat[q], None, 0, tc_n // 2))
            if b == 11:
                tail_norm(0)
            elif b == 12:
                tail_zr(0)
            elif b == 13:
                tail_p_out(0)
            prev = (b, [th0, th1])
        at2_last = stage_et_exp(prev[0], prev[1])
        q = prev[0] // 2
        stage_ctx_finish(q, stage_ctx_half(q, at2_last, pair_xnat.pop(q), None, 0, tc_n))
        keepalive(0)
        tail_norm(1)
        keepalive(1)
        tail_zr(1)
        keepalive(2)
        tail_p_out(1)

    if split_waits:
        split_multi_waits(nc)
    return nc


def _host_prep(inputs, h_tm, V_a, W_a, U_a, b_a, C_z, W_z, b_z, C_r, W_r, b_r,
               C_p, U_p, b_p):
    """Fold everything not depending on x_seq into small per-core tensors."""
    wxpb = h_tm @ W_a + b_a                                # [B, U]
    g_z0 = h_tm @ W_z + inputs @ C_z[:IN_DIM] + b_z        # [B, U]
    g_r0 = h_tm @ W_r + inputs @ C_r[:IN_DIM] + b_r
    g_p0 = inputs @ C_p[:IN_DIM] + b_p
    # uav[p, c, :]: ua row (c*128+p) in cols 0..U-1, va[c*128+p] in col U --
    # one packed DMA with >=512 contiguous bytes per partition.
    uav = np.zeros((P, EC, U + 16), dtype=NPF8)
    uav[:, :, :U] = U_a.astype(NPF8).reshape(EC, P, U).transpose(1, 0, 2)
    uav[:, :, U] = V_a.astype(NPF8).reshape(EC, P).T
    shared = {
        "uav": np.ascontiguousarray(uav),
        "cz": np.ascontiguousarray(C_z[IN_DIM:].astype(BF16)),
        "cr": np.ascontiguousarray(C_r[IN_DIM:].astype(BF16)),
        "cp": np.ascontiguousarray(C_p[IN_DIM:].astype(BF16)),
        "up": np.ascontiguousarray(U_p.astype(BF16)),
        "ident": np.eye(P, dtype=np.float32),
    }
    per_core = []
    for c in range(N_CORES):
        s = slice(c * BS, (c + 1) * BS)
        # fsm[p, 0:2]: wxpb^T; [p, 2:4]: h^T; [p, 4:10]: g0^T for z, r, p --
        # all [u%128 -> p, u//128 -> chunk, b] layouts packed in one tensor.
        def chunked(m):  # [bs, U] -> [P, UC, bs]
            return m.T.astype(np.float32).reshape(UC, P, BS).transpose(1, 0, 2)
        fsm = np.concatenate(
            [chunked(wxpb[s]), chunked(h_tm[s]), chunked(g_z0[s]),
             chunked(g_r0[s]), chunked(g_p0[s])], axis=1
        )
        per_core.append({"fsm": np.ascontiguousarray(fsm), **shared})
    return per_core


def _prep_x(x_core):
    """Pre-tile one core's x [bs, TE, U] into both fp8 layouts."""
    xb = x_core.astype(NPF8)
    tc_n = TE // P
    # xnat[q, p, tc, j, e] = x[2q+j, tc*128+p, e]  (pair-interleaved)
    xnat = np.ascontiguousarray(
        xb.reshape(BS // 2, 2, tc_n, P, U).transpose(0, 3, 2, 1, 4)
    )
    # xtr[b, p, ec, t] = x[b, t, ec*128+p]
    xtr = np.ascontiguousarray(
        xb.reshape(BS, TE, EC, P).transpose(0, 3, 2, 1)
    )
    return xnat, xtr


def build_in_maps(all_inputs):
    """Full host prep: dict of the reference's 16 inputs -> per-core in_maps."""
    args = {k: np.asarray(v, dtype=np.float32) for k, v in all_inputs.items()
            if k != "x_seq"}
    x_seq = np.asarray(all_inputs["x_seq"], dtype=np.float32)
    per_core = _host_prep(**args)
    in_maps = []
    for c in range(N_CORES):
        m = dict(per_core[c])
        m["xnat"], m["xtr"] = _prep_x(x_seq[c * BS : (c + 1) * BS])
        in_maps.append(m)
    return in_maps


def kernel(inputs, h_tm, x_seq, V_a, W_a, U_a, b_a, C_z, W_z, b_z,
           C_r, W_r, b_r, C_p, U_p, b_p):
    from concourse.bass_utils import run_bass_kernel_spmd

    in_maps = build_in_maps(dict(
        inputs=inputs, h_tm=h_tm, x_seq=x_seq, V_a=V_a, W_a=W_a, U_a=U_a,
        b_a=b_a, C_z=C_z, W_z=W_z, b_z=b_z, C_r=C_r, W_r=W_r, b_r=b_r,
        C_p=C_p, U_p=U_p, b_p=b_p))
    nc = build_nc()
    res = run_bass_kernel_spmd(nc, in_maps, core_ids=list(range(N_CORES)))
    return np.concatenate([res.results[c]["ht"] for c in range(N_CORES)], axis=0)


# revision 26
# speedup vs baseline: 1.2754x; 1.1031x over previous
"""AttentionRNNCell Trainium2 kernel (v4).

Math (per batch row b):
  et[t]  = V_a . tanh( (h W_a + b_a) + x[t] U_a )        t in [0, TE)
  at     = exp(et);  s = sum(at)
  ctx    = (sum_t at[t] x[t]) / s
  zt     = sigmoid(h W_z + [inp, ctx] C_z + b_z)
  rt     = sigmoid(h W_r + [inp, ctx] C_r + b_r)
  tht    = tanh((rt*h) U_p + [inp, ctx] C_p + b_p)
  ht     = (1-zt)*h + zt*tht
Distribution: data-parallel over batch B=128 across 8 cores (16 rows each).
Host ships x twice in fp8, pre-tiled in both layouts the PE needs
(xnat: t on partitions for ctx; xtr: e on partitions for uxpb), and folds
everything not depending on x_seq into small per-core tensors.

v4 vs v3 (trace-driven):
  - rows 0/1 xtr go first on the gpsimd (SWDGE) ring in half-row DMAs;
    rows 2-15 ship as PAIR tiles in one DMA each (8KB/partition contiguous
    -> 8KB descriptors; the v3 half-row split produced 1KB descriptors and
    the HWDGE ring drained at only ~76GB/s).
  - xtr pair pool bufs=4 (8 rows of lookahead) so the DMA stream runs at
    ring rate instead of being throttled to compute pace by pool WAR deps
    (v3's row 15 data landed at ~100us, stretching the whole kernel).
  - a dummy ACT right after the preamble pulls the one-time ACT table load
    (~2.7us) off the first-tanh critical path.
  - ctx is transposed + staged per PAIR right after its matmul (DVE copy
    of the [2,256] psum, two tiny PE transposes, DVE copies into ctxn) --
    no SBUF->SBUF ctx_rows DMAs, whose ~2.4us completion latency sat on
    the v3 tail.  Normalization by 1/s happens once per half-batch.
  - gate weights / ctxn / rh are bf16 (fast FWL ldweights; v3's fp32 gate
    LDWEIGHTS were 333ns each) and each gate's two u-chunks share one
    [P,2,8] psum tile -> one ACT per gate (3 per half instead of 6 chains).
  - sigmoid = 1/(1+exp(-x)) on DVE keeps the whole kernel inside the one
    exp_and_others ACT table set.
"""

from contextlib import ExitStack

import numpy as np
import ml_dtypes

import concourse.bass as bass
import concourse.mybir as mybir
import concourse.tile as tile

BF16 = ml_dtypes.bfloat16
NPF8 = ml_dtypes.float8_e4m3
F32 = mybir.dt.float32
BF = mybir.dt.bfloat16
F8 = mybir.dt.float8e4
DR = mybir.MatmulPerfMode.DoubleRow
AF = mybir.ActivationFunctionType
AX = mybir.AxisListType
ALU = mybir.AluOpType

B, TE, U, IN_DIM = 128, 2048, 256, 256
N_CORES = 8
BS = B // N_CORES  # 16 batch rows per core
P = 128
EC = U // P  # e-chunks (2)
UC = U // P  # u-chunks (2)
HB = BS // 2  # rows per tail half (8)


def split_multi_waits(nc, max_waits=1):
    """This container's walrus rejects instructions carrying more than one
    sync wait. Hoist extra waits onto standalone same-engine NoOps inserted
    immediately before the offending instruction (semantically identical:
    the engine blocks on each wait in order before executing it)."""
    n_new = 0
    for f in nc.m.functions:
        for blk in f.blocks:
            new_insts = []
            for inst in blk.instructions:
                si = inst.sync_info
                waits = list(si.on_wait) if si and si.on_wait else []
                if len(waits) > max_waits:
                    for w in waits[:-max_waits]:
                        nop = mybir.InstNoOp(
                            name=f"{inst.name}-hw{n_new}", ins=[], outs=[]
                        )
                        nop.engine = inst.engine
                        nop.sync_info = mybir.SyncInfo(on_wait=[w], on_update=[])
                        new_insts.append(nop)
                        n_new += 1
                    si.on_wait = waits[-max_waits:]
                new_insts.append(inst)
            blk.instructions = new_insts
    return n_new


def build_nc(bs=BS, te=TE, split_waits=True):
    tc_n = te // P      # 128-col t-chunks (16)
    th_n = 2            # t halves
    t_half = te // th_n
    tq_n = t_half // P  # 128-col chunks per half (8)
    n_mm = min(512, t_half)

    nc = bass.Bass()
    xnat_d = nc.declare_dram_parameter("xnat", [bs // 2, P, tc_n, 2, U], F8, isOutput=False)
    xtr_d = nc.declare_dram_parameter("xtr", [bs, P, EC, te], F8, isOutput=False)
    # Small weights ship pre-permuted and packed so every DMA moves >=512
    # contiguous bytes per partition (tiny strided descriptors -- e.g. va as
    # 256 one-byte RMW descriptors -- took >20us on the HWDGE ring and sat
    # in front of the first tanh's bias).
    uav_d = nc.declare_dram_parameter("uav", [P, EC, U + 16], F8, isOutput=False)
    fsm_d = nc.declare_dram_parameter("fsm", [P, 10, bs], F32, isOutput=False)
    cz_d = nc.declare_dram_parameter("cz", [U, U], BF, isOutput=False)
    cr_d = nc.declare_dram_parameter("cr", [U, U], BF, isOutput=False)
    cp_d = nc.declare_dram_parameter("cp", [U, U], BF, isOutput=False)
    up_d = nc.declare_dram_parameter("up", [U, U], BF, isOutput=False)
    id_d = nc.declare_dram_parameter("ident", [P, P], F32, isOutput=False)
    ht_d = nc.declare_dram_parameter("ht", [bs, U], F32, isOutput=True)

    with tile.TileContext(nc) as tc, ExitStack() as ctx:
        singles = ctx.enter_context(tc.tile_pool(name="singles", bufs=1))
        xnat_p = ctx.enter_context(tc.tile_pool(name="xnat", bufs=2))
        xtr_p = ctx.enter_context(tc.tile_pool(name="xtr", bufs=4))
        tanh_p = ctx.enter_context(tc.tile_pool(name="tanh", bufs=8))
        small_p = ctx.enter_context(tc.tile_pool(name="small", bufs=4))
        uxpb_ps = ctx.enter_context(tc.tile_pool(name="uxpbps", bufs=3, space="PSUM"))
        et_ps = ctx.enter_context(tc.tile_pool(name="etps", bufs=1, space="PSUM"))
        tail_ps = ctx.enter_context(tc.tile_pool(name="tailps", bufs=1, space="PSUM"))

        # ---- weights / small per-core tensors ----
        uav_sb = singles.tile([P, EC, U + 16], F8)  # ua cols 0..255, va col 256
        # (k-tile stride padded to 272 = 17*16: dual-fp8 ldweights needs %16==0)
        fsm_sb = singles.tile([P, 10, bs], F32)     # wxpb 0:2, hT 2:4, g0 4:10
        ua_sb = uav_sb
        va_sb = uav_sb[:, :, U : U + 1]

        def load_first_weights():
            # Head of the sync HWDGE ring: ~150KB, lands ~1us after flow
            # starts, ahead of row 0's x.
            nc.sync.dma_start(out=uav_sb, in_=uav_d[:, :, :])
            nc.sync.dma_start(out=fsm_sb, in_=fsm_d[:, :, :])

        gate_w = {}
        for name in ("cz", "cr", "cp", "up"):
            gate_w[name] = singles.tile([P, EC, U], BF, name=f"{name}_sb")
        id_sb = singles.tile([P, P], F32)
        idb_sb = singles.tile([P, P], BF)
        ones_sb = singles.tile([P, P], F32)
        nc.vector.memset(ones_sb, 1.0)
        ones8 = singles.tile([P, 512], F8)
        nc.vector.memset(ones8, 1.0)

        def load_tail_weights():
            # sync ring, right after row 1: needed from ~iteration 3's ctx.
            nc.sync.dma_start(out=id_sb, in_=id_d[:, :])
            nc.vector.tensor_copy(idb_sb, id_sb)

        def load_gate_weights():
            # sync ring, after pair (6,7)'s xtr (bf16, 128KB each).
            for name, d in (("cz", cz_d), ("cr", cr_d), ("cp", cp_d), ("up", up_d)):
                nc.sync.dma_start(out=gate_w[name], in_=d[:, :].rearrange("(c p) u -> p c u", p=P))

        expsum_all = singles.tile([P, bs], F32)
        # Unnormalized ctx^T columns, staged per pair via tiny PE transposes
        # (no SBUF->SBUF DMA). Normalized in-place per half-batch.
        ctxn = singles.tile([P, EC, bs], BF)
        # Block-diagonal at tiles for the paired-ctx DoubleRow: slot [j, m]
        # holds row (2q+j)'s at iff j == m, else stays the zero written once
        # here. Two tiles ping-pong across pairs. [p, j, tc, m] layout: the
        # k-tile (j) stride is tc_n*2 bytes (dual-fp8 ldweights needs >=16B).
        at2_tiles = []
        for i in range(2):
            at2 = singles.tile([P, 2, tc_n, 2], F8, name=f"at2_{i}")
            nc.vector.memset(at2, 0.0)
            at2_tiles.append(at2)

        # ---- ACT table preload + HAM warmup, both during the initial DMA
        # wait: the weight DMAs issue first on the scalar ring, then a dummy
        # ACT pulls the one-time exp_and_others table load off the
        # first-tanh critical path; ~2.6us of dummy matmuls get the PE
        # clock-gate warming before row 0's data lands.
        load_first_weights()
        actwarm = small_p.tile([P, 1], F32, name="actwarm")
        nc.scalar.activation(out=actwarm, in_=ones_sb[:, 0:1], func=AF.Tanh)
        warm = uxpb_ps.tile([P, 512], F32, tag="ux", name="warm")
        for _ in range(6):
            nc.tensor.matmul(out=warm, lhsT=ones8[:, 0:P], rhs=ones8)

        # ---- streaming stages ----
        pend_pair = {}
        defer_xnat = []

        def stage_dma(b):
            # xtr first in each iteration's ring order: it gates compute one
            # row ahead, while xnat is only needed two iterations later.
            if b < 2:
                # Rows 0/1: one full-row DMA each, right behind the packed
                # weights on the sync ring (4KB/partition contiguous).
                xt = singles.tile([P, EC, te], F8, name=f"xt{b}")
                nc.sync.dma_start(out=xt, in_=xtr_d[b])
            elif b % 2 == 0:
                # One DMA per pair: per-partition-contiguous 2x4KB source
                # blocks -> big descriptors, full HWDGE ring rate.
                xp = xtr_p.tile([P, 2, EC, te], F8, tag="xt", name=f"xt{b}")
                nc.sync.dma_start(
                    out=xp, in_=xtr_d[b : b + 2].rearrange("j p c t -> p j c t")
                )
                pend_pair[b // 2] = xp
                xt = xp[:, 0]
            else:
                xt = pend_pair.pop(b // 2)[:, 1]
            x_nat = None
            if b % 2 == 0:
                x_nat = xnat_p.tile([P, tc_n, 2, U], F8, tag="xnat", name=f"xnat{b}")
                if b == 0:
                    # Deferred behind rows 0/1 + id on the sync ring.
                    defer_xnat.append(x_nat)
                else:
                    # Pair 1 rides the sync ring too: the SDMA engines
                    # round-robin between rings at packet granularity, and
                    # SWDGE's 8KB descriptors would starve the startup-
                    # critical transfers.  From pair 2 on, the xnat pool's
                    # WAR dep (bufs=2) holds the SWDGE ring back until the
                    # matching ctx is done.
                    eng = nc.sync if b == 2 else nc.gpsimd
                    eng.dma_start(out=x_nat, in_=xnat_d[b // 2])
            if b == 1:
                nc.sync.dma_start(out=defer_xnat[0], in_=xnat_d[0])
            return x_nat, xt

        def stage_uxpb_th(b, th, xt):
            # uxpb: out[u, t] = sum_e ua[e, u] * xt[e, t] -- fp8 DoubleRow
            # contracts both e-chunks in one matmul. tanh (per-partition
            # bias) -> SBUF fp8 [u, uc, t] tiles for the et DoubleRow.
            tanh_t = tanh_p.tile([P, UC, t_half], F8, tag="tanh", name=f"th{b}_{th}")
            for uc in range(UC):
                ux = uxpb_ps.tile([P, t_half], F32, tag="ux", name=f"ux{b}{th}{uc}")
                for n0 in range(0, t_half, n_mm):
                    nc.tensor.matmul(
                        out=ux[:, n0 : n0 + n_mm],
                        lhsT=ua_sb[:, :, uc * P : (uc + 1) * P],
                        rhs=xt[:, :, th * t_half + n0 : th * t_half + n0 + n_mm],
                        perf_mode=DR,
                    )
                nc.scalar.activation(
                    out=tanh_t[:, uc, :], in_=ux, func=AF.Tanh,
                    bias=fsm_sb[:, uc, b : b + 1],
                )
            return tanh_t

        def stage_et_exp(b, tanh_ts):
            et = et_ps.tile([P, tc_n], F32, tag="etp", name=f"et{b}")
            for th in range(th_n):
                for tq in range(tq_n):
                    nc.tensor.matmul(
                        out=et[:, th * tq_n + tq : th * tq_n + tq + 1],
                        lhsT=tanh_ts[th][:, :, tq * P : (tq + 1) * P],
                        rhs=va_sb,
                        perf_mode=DR,
                    )
            # exp lands on the diagonal slot of the pair's block-diag tile;
            # expsum via DVE reduce of that slot (no ACT accumulator read).
            j = b % 2
            at2 = at2_tiles[(b // 2) % 2]
            nc.scalar.activation(out=at2[:, j, :, j], in_=et, func=AF.Exp)
            nc.vector.tensor_reduce(
                out=expsum_all[:, b : b + 1], in_=at2[:, j, :, j],
                axis=AX.X, op=ALU.add,
            )
            return at2

        def stage_ctx_half(q, at2, x_nat, cps, lo, hi):
            # Paired ctx: block-diagonal at2 on the two k-tiles against the
            # pair-interleaved x tile -> out[m, e] = row (2q+m)'s ctx partial.
            # Issued as two 8-matmul chunks in consecutive iterations so PE
            # load stays balanced against the ACT tanh pace.
            if cps is None:
                cps = tail_ps.tile([2, U], F32, tag="tail", name=f"cps{q}")
            for tcc in range(lo, hi):
                nc.tensor.matmul(
                    out=cps,
                    lhsT=at2[:, :, tcc, :],
                    rhs=x_nat[:, tcc, :, :],
                    start=(tcc == 0),
                    stop=(tcc == tc_n - 1),
                    perf_mode=DR,
                )
            return cps

        def stage_ctx_finish(q, cps):
            # Stage straight into ctxn columns via tiny PE transposes (bf16),
            # unnormalized; 1/s is applied once per half-batch.
            stg = small_p.tile([2, U], BF, tag="ctxstg", name=f"stg{q}")
            nc.vector.tensor_copy(stg, cps)
            for e in range(EC):
                tp = tail_ps.tile([P, 2], BF, tag="tail", name=f"ctxT{q}{e}")
                nc.tensor.transpose(tp, stg[:, e * P : (e + 1) * P], idb_sb[0:2, 0:2])
                nc.vector.tensor_copy(ctxn[:, e, 2 * q : 2 * q + 2], tp)

        # ---- tail: per half-batch of 8 rows (h=0 pipelined into the
        # stream at iterations 11-13, h=1 after the loop) ----
        recips_h = [None, None]
        zt_h = [None, None]
        rh_h = [None, None]

        def tail_s(h):
            rows = slice(HB * h, HB * h + HB)
            s_ps = tail_ps.tile([P, HB], F32, tag="tail", name=f"sps{h}")
            nc.tensor.matmul(out=s_ps, lhsT=ones_sb, rhs=expsum_all[:, rows])
            rec = small_p.tile([P, HB], F32, name=f"recip{h}")
            nc.vector.reciprocal(rec, s_ps)
            recips_h[h] = rec

        def tail_mul(h):
            rows = slice(HB * h, HB * h + HB)
            for e in range(EC):
                nc.vector.tensor_mul(ctxn[:, e, rows], ctxn[:, e, rows], recips_h[h])

        def _gate_psum(h, parts, name):
            # One [P, UC, HB] psum tile accumulating all (weight, rhs) pairs
            # for both u-chunks -> a single ACT covers the whole gate.
            rows = slice(HB * h, HB * h + HB)
            g = tail_ps.tile([P, UC, HB], F32, tag="tail", name=name)
            for uc in range(UC):
                i = 0
                for w_sb, rhs_fn in parts:
                    for e in range(EC):
                        nc.tensor.matmul(
                            out=g[:, uc, :],
                            lhsT=w_sb[:, e, uc * P : (uc + 1) * P],
                            rhs=rhs_fn(e),
                            start=(i == 0),
                            stop=(i == len(parts) * EC - 1),
                        )
                        i += 1
            return g

        def _tail_gate(h, gi, wname):
            # One gate chain: MMs -> +g0 -> sigmoid via exp -> (rh mul).
            rows = slice(HB * h, HB * h + HB)
            g = _gate_psum(h, [(gate_w[wname], lambda e: ctxn[:, e, rows])], f"g{wname}{h}")
            tmp = small_p.tile([P, UC, HB], F32, tag="gtmp", name=f"t{wname}{h}")
            nc.vector.tensor_add(tmp, g, fsm_sb[:, 4 + 2 * gi : 6 + 2 * gi, rows])
            # sigmoid(v) = 1/(1+exp(-v)): stays in the exp table set.
            ex = small_p.tile([P, UC, HB], F32, tag="gtmp", name=f"e{wname}{h}")
            nc.scalar.activation(out=ex, in_=tmp, func=AF.Exp, scale=-1.0)
            nc.vector.tensor_scalar_add(tmp, ex, 1.0)
            if gi == 0:
                zt = small_p.tile([P, UC, HB], F32, name=f"zt{h}")
                nc.vector.reciprocal(zt, tmp)
                zt_h[h] = zt
            else:
                rt = small_p.tile([P, UC, HB], F32, tag="gtmp", name=f"rt{h}")
                nc.vector.reciprocal(rt, tmp)
                rh = small_p.tile([P, UC, HB], BF, name=f"rh{h}")
                nc.vector.tensor_mul(rh, rt, fsm_sb[:, 2:4, rows])
                rh_h[h] = rh

        def tail_z(h):
            _tail_gate(h, 0, "cz")

        def tail_r(h):
            _tail_gate(h, 1, "cr")

        def tail_p_out(h):
            rows = slice(HB * h, HB * h + HB)
            zt, rh = zt_h[h], rh_h[h]
            g = _gate_psum(
                h,
                [(gate_w["up"], lambda e: rh[:, e, :]),
                 (gate_w["cp"], lambda e: ctxn[:, e, rows])],
                f"gp{h}",
            )
            gtmp = small_p.tile([P, UC, HB], F32, tag="gtmp", name=f"gt{h}")
            nc.vector.tensor_add(gtmp, g, fsm_sb[:, 8:10, rows])
            tht = small_p.tile([P, UC, HB], F32, tag="gtmp", name=f"tht{h}")
            nc.scalar.activation(out=tht, in_=gtmp, func=AF.Tanh)
            # ht^T = h^T + zt^T*(tht^T - h^T)
            nc.vector.tensor_sub(tht, tht, fsm_sb[:, 2:4, rows])
            nc.vector.tensor_mul(tht, tht, zt)
            nc.vector.tensor_add(tht, tht, fsm_sb[:, 2:4, rows])
            stg = small_p.tile([HB, U], F32, name=f"htstg{h}")
            for uc in range(UC):
                tp = tail_ps.tile([HB, P], F32, tag="tail", name=f"htp{h}{uc}")
                nc.tensor.transpose(tp, tht[:, uc, :], id_sb)
                nc.vector.tensor_copy(stg[:, uc * P : (uc + 1) * P], tp)
            nc.sync.dma_start(out=ht_d[rows, :], in_=stg)

        def keepalive(i):
            # One matmul to reset the PE clock-gate's idle window during the
            # latency-bound tail (else it re-throttles to 1.2GHz mid-tail).
            ka = uxpb_ps.tile([P, 512], F32, tag="ux", name=f"ka{i}")
            nc.tensor.matmul(out=ka, lhsT=ones8[:, 0:P], rhs=ones8)

        # ---- main loop, software-pipelined one row deep ----
        prev = None  # (b, tanh_ts)
        pair_xnat = {}
        pend_ctx = None  # (q, cps) with 8 of 16 matmuls issued
        for b in range(bs):
            x_nat, xt = stage_dma(b)
            if x_nat is not None:
                pair_xnat[b // 2] = x_nat
            if b == 1:
                load_tail_weights()
            if b == 6:
                load_gate_weights()
            th0 = stage_uxpb_th(b, 0, xt)
            # Tail work for rows 0-7, spread one small stage per iteration,
            # placed right after th0 so its tiny matmuls complete well
            # before its ACT slot comes up (no ACT head-of-line stall).
            if b == 11:
                tail_s(0)
                tail_mul(0)
            elif b == 12:
                tail_z(0)
            elif b == 13:
                tail_r(0)
            elif b == 14:
                tail_p_out(0)
            if pend_ctx is not None:
                q, cps = pend_ctx
                stage_ctx_finish(q, stage_ctx_half(q, at2_tiles[q % 2], pair_xnat.pop(q), cps, tc_n // 2, tc_n))
                pend_ctx = None
            at2_prev = None
            if prev is not None:
                at2_prev = stage_et_exp(prev[0], prev[1])
                pb = prev[0]
            th1 = stage_uxpb_th(b, 1, xt)
            if prev is not None and pb % 2 == 1 and pb < bs - 1:
                q = pb // 2
                pend_ctx = (q, stage_ctx_half(q, at2_prev, pair_xnat[q], None, 0, tc_n // 2))
            prev = (b, [th0, th1])
        at2_last = stage_et_exp(prev[0], prev[1])
        tail_s(1)
        q = prev[0] // 2
        stage_ctx_finish(q, stage_ctx_half(q, at2_last, pair_xnat.pop(q), None, 0, tc_n))
        keepalive(0)
        tail_mul(1)
        tail_z(1)
        keepalive(1)
        tail_r(1)
        keepalive(2)
        tail_p_out(1)

    if split_waits:
        split_multi_waits(nc)
    return nc


def _host_prep(inputs, h_tm, V_a, W_a, U_a, b_a, C_z, W_z, b_z, C_r, W_r, b_r,
               C_p, U_p, b_p):
    """Fold everything not depending on x_seq into small per-core tensors."""
    wxpb = h_tm @ W_a + b_a                                # [B, U]
    g_z0 = h_tm @ W_z + inputs @ C_z[:IN_DIM] + b_z        # [B, U]
    g_r0 = h_tm @ W_r + inputs @ C_r[:IN_DIM] + b_r
    g_p0 = inputs @ C_p[:IN_DIM] + b_p
    # uav[p, c, :]: ua row (c*128+p) in cols 0..U-1, va[c*128+p] in col U --
    # one packed DMA with >=512 contiguous bytes per partition.
    uav = np.zeros((P, EC, U + 16), dtype=NPF8)
    uav[:, :, :U] = U_a.astype(NPF8).reshape(EC, P, U).transpose(1, 0, 2)
    uav[:, :, U] = V_a.astype(NPF8).reshape(EC, P).T
    shared = {
        "uav": np.ascontiguousarray(uav),
        "cz": np.ascontiguousarray(C_z[IN_DIM:].astype(BF16)),
        "cr": np.ascontiguousarray(C_r[IN_DIM:].astype(BF16)),
        "cp": np.ascontiguousarray(C_p[IN_DIM:].astype(BF16)),
        "up": np.ascontiguousarray(U_p.astype(BF16)),
        "ident": np.eye(P, dtype=np.float32),
    }
    per_core = []
    for c in range(N_CORES):
        s = slice(c * BS, (c + 1) * BS)
        # fsm[p, 0:2]: wxpb^T; [p, 2:4]: h^T; [p, 4:10]: g0^T for z, r, p --
        # all [u%128 -> p, u//128 -> chunk, b] layouts packed in one tensor.
        def chunked(m):  # [bs, U] -> [P, UC, bs]
            return m.T.astype(np.float32).reshape(UC, P, BS).transpose(1, 0, 2)
        fsm = np.concatenate(
            [chunked(wxpb[s]), chunked(h_tm[s]), chunked(g_z0[s]),
             chunked(g_r0[s]), chunked(g_p0[s])], axis=1
        )
        per_core.append({"fsm": np.ascontiguousarray(fsm), **shared})
    return per_core


def _prep_x(x_core):
    """Pre-tile one core's x [bs, TE, U] into both fp8 layouts."""
    xb = x_core.astype(NPF8)
    tc_n = TE // P
    # xnat[q, p, tc, j, e] = x[2q+j, tc*128+p, e]  (pair-interleaved)
    xnat = np.ascontiguousarray(
        xb.reshape(BS // 2, 2, tc_n, P, U).transpose(0, 3, 2, 1, 4)
    )
    # xtr[b, p, ec, t] = x[b, t, ec*128+p]
    xtr = np.ascontiguousarray(
        xb.reshape(BS, TE, EC, P).transpose(0, 3, 2, 1)
    )
    return xnat, xtr


def build_in_maps(all_inputs):
    """Full host prep: dict of the reference's 16 inputs -> per-core in_maps."""
    args = {k: np.asarray(v, dtype=np.float32) for k, v in all_inputs.items()
            if k != "x_seq"}
    x_seq = np.asarray(all_inputs["x_seq"], dtype=np.float32)
    per_core = _host_prep(**args)
    in_maps = []
    for c in range(N_CORES):
        m = dict(per_core[c])
        m["xnat"], m["xtr"] = _prep_x(x_seq[c * BS : (c + 1) * BS])
        in_maps.append(m)
    return in_maps


def kernel(inputs, h_tm, x_seq, V_a, W_a, U_a, b_a, C_z, W_z, b_z,
           C_r, W_r, b_r, C_p, U_p, b_p):
    from concourse.bass_utils import run_bass_kernel_spmd

    in_maps = build_in_maps(dict(
        inputs=inputs, h_tm=h_tm, x_seq=x_seq, V_a=V_a, W_a=W_a, U_a=U_a,
        b_a=b_a, C_z=C_z, W_z=W_z, b_z=b_z, C_r=C_r, W_r=W_r, b_r=b_r,
        C_p=C_p, U_p=U_p, b_p=b_p))
    nc = build_nc()
    res = run_bass_kernel_spmd(nc, in_maps, core_ids=list(range(N_CORES)))
    return np.concatenate([res.results[c]["ht"] for c in range(N_CORES)], axis=0)
